# revision 1
# baseline (speedup 1.0000x reference)
"""Trainium2 Bass kernel v2 for the 7-layer binarized CNN (nn_MCNET).

Data parallel over 8 cores (8 images each). Per core:
- L0 (3->4, fp32 input): input exactly decomposed into 3 bf16 terms
  (hi/lo/lolo, residual ~2^-25) on the idle Pool engine one image ahead;
  27 accumulating bf16 matmuls (32 row-bands x 8 rows, block-diagonal
  weights); per-psum-chunk ACT Sign -> bf16, DVE 2x2 maxpool writes fp8
  directly into A1's banded layout.
- L1..L6: fp8e4 DoubleRow matmuls. Activations live in per-layer banded
  buffers A_l: G bands x cin channels on 128 partitions, rows contiguous at
  stride 127 (127 % 16 == 15, so tap pairs (ki,kj)->(ki+1,kj+1) have ktile
  stride 128, a legal DoubleRow step). Tap-paired layers run 6 DoubleRow
  passes instead of 9; L5 (cin=64) k-splits channels across two 4320-byte
  slabs instead. Band nesting is chosen so every PSUM evacuation is an
  identity-partition ACT/DVE op (sign == clip for the even-integer sums),
  and inter-band halos are ONE contiguous partition-shifted SBUF DMA per
  layer. All DMAs issue from the SP (sync) engine -> HWDGE.
"""
import sys, os, dataclasses
sys.path.insert(0, '/opt/trn_rl_repo')
import numpy as np

CH = [(3, 4), (4, 8), (8, 16), (16, 32), (32, 64), (64, 32), (32, 2)]
HIN = [256, 127, 125, 123, 121, 119, 117]
HOUT = [h - 2 for h in HIN]
G = [32, 32, 16, 8, 4, 4, 4]         # in-bands per layer
BB = [8, 4, 8, 16, 32, 32, 32]       # nominal in-band rows
WP = 127                              # fp8 row stride (127 % 16 == 15)
SL5 = 4320                            # A5 slab stride (34*127=4318 -> pad to %16)
NIMG = 8
# tap pairs for t2 (tap-paired DoubleRow): ktile delta = 127*dki + dkj = 128
PAIRS = [((0, 0), (1, 1)), ((0, 1), (1, 2)), ((1, 0), (2, 1)),
         ((0, 2), None), ((2, 0), None), ((2, 2), None)]
# weight block column offsets in WF8
NB = [0, 12, 12, 12, 12, 9, 6]        # lhsT blocks per layer (l1..l6 used)
OFF = {}
_c = 0
for _l in range(1, 7):
    OFF[_l] = _c
    _c += NB[_l] * (256 if _l < 6 else 32)
WF8_COLS = _c
A_ROWS = [0, 6, 10, 18, 34, 0, 34]    # stored rows per band (A5 special)
A_COLS = [0] + [A_ROWS[l] * WP + 384 for l in range(1, 7)]
A_COLS[5] = 2 * SL5 + 384


def build_program():
    import concourse.bass as bass
    import concourse.mybir as mybir
    dt = mybir.dt
    AF = mybir.ActivationFunctionType
    PM = mybir.MatmulPerfMode
    ALU = mybir.AluOpType

    nc = bass.Bass("TRN2", target_bir_lowering=False)
    x = nc.dram_tensor("x", (NIMG, 3, 256, 256), dt.float32, kind="ExternalInput")
    w0f = nc.dram_tensor("w0f", (96, 9 * 128), dt.bfloat16, kind="ExternalInput")
    wf8 = nc.dram_tensor("wf8", (128, WF8_COLS), dt.float8e4, kind="ExternalInput")
    y = nc.dram_tensor("y", (NIMG, 2 * 115 * 115), dt.float32, kind="ExternalOutput")

    ctxs = []
    def alloc(cm):
        ctxs.append(cm)
        return cm.__enter__()

    WT0 = alloc(nc.sbuf_tensor("WT0", [128, 9 * 128], dt.bfloat16))
    WF8 = alloc(nc.sbuf_tensor("WF8", [128, WF8_COLS], dt.float8e4))
    A0 = alloc(nc.sbuf_tensor("A0", [128, 2 * 10 * 256], dt.float32))
    H = alloc(nc.sbuf_tensor("H", [128, 2 * 3 * 2560], dt.bfloat16))
    R1 = alloc(nc.sbuf_tensor("R1", [128, 2560], dt.float32))
    R2 = alloc(nc.sbuf_tensor("R2", [128, 2560], dt.float32))
    A = [None] * 7
    for l in range(1, 7):
        A[l] = alloc(nc.sbuf_tensor(f"A{l}", [128, A_COLS[l]], dt.float8e4))
    T0B = alloc(nc.sbuf_tensor("T0B", [128, 2032], dt.bfloat16))
    T1B = alloc(nc.sbuf_tensor("T1B", [128, 4 * 254], dt.bfloat16))
    OUTB = alloc(nc.sbuf_tensor("OUTB", [128, 32 * 115], dt.float32))
    P = [alloc(nc.psum_tensor(f"P{i}", [128, 2048], dt.float32)) for i in range(2)]
    sem = {n: alloc(nc.semaphore(name=n)) for n in
           ['sdma', 'spe', 'sact', 'sdve', 'sgp', 'sin0', 'sin1', 'swf',
            'sh1', 'sh2', 'sh3', 'sh4', 'sh5', 'sh6', 'sout']}

    def walk(E, me):
        cnt = {'dma': 0, 'pe': 0, 'act': 0, 'dve': 0, 'gp': 0}
        last_wait = {}

        def wait(eng, semn, val):
            if val is None or val <= 0:
                return
            k = (eng, semn)
            if last_wait.get(k, -1) >= val:
                return
            last_wait[k] = val
            if eng == me:
                E.wait_ge(sem[semn], val)

        def emit(eng, fn):
            if eng == me:
                return fn()
            return None

        def inc(inst, semn, v):
            if inst is not None:
                inst.then_inc(sem[semn], v)

        # ---- init: memsets ----
        # A0: only band 31's rows 8,9 are never DMA-written (both slots).
        i = emit('gp', lambda: nc.gpsimd.memset(A0[64:96, 2048:2560], 0.0))
        cnt['gp'] += 1
        inc(i, 'sgp', 1)
        i = emit('gp', lambda: nc.gpsimd.memset(A0[64:96, 4608:5120], 0.0))
        cnt['gp'] += 1
        inc(i, 'sgp', 1)
        for l in range(1, 7):
            i = emit('gp', lambda l=l:
                     nc.gpsimd.memset(A[l][0:128, 0:A_COLS[l]], 0.0))
            cnt['gp'] += 1
            inc(i, 'sgp', 1)
        NMEMSET = cnt['gp']
        MS_A = {0: 2, 1: 3, 2: 4, 3: 5, 4: 6, 5: 7, 6: 8}  # sgp count when A_l ready
        # ---- weight DMA: WT0 first (L0); WF8 deferred until after in(0/1) ----
        i = emit('sp', lambda: nc.sync.dma_start(WT0[0:96, :], w0f[:]))
        inc(i, 'sdma', 16)

        slot_free = [None, None]
        a0_free = [None, None]
        h_free = [None, None]
        vmax_prev = None            # sdve count of prev img vmax (T0B free)
        hcnt = {l: 0 for l in range(1, 7)}   # per-halo-sem cumulative counts
        out_cnt = 0
        tile_g = 0

        def pe_tile_begin(slot, waits_other=(), layer=1):
            # common PE-tile prologue: memsets + weights + input-ready + slot
            if layer == 0:
                wait('pe', 'sgp', 2)        # A0 tail memsets
                wait('pe', 'sdma', 16)      # WT0
            else:
                wait('pe', 'sgp', MS_A[layer])   # A_layer memset done
                wait('pe', 'swf', 16)       # WF8
            for sname, v in waits_other:
                wait('pe', sname, v)
            if slot_free[slot] is not None:
                wait('pe', slot_free[slot][0], slot_free[slot][1])

        def dr_matmul(PS, psoff, lhs_col, lhs_m, rhs_buf, rhs_off, rhs_delta,
                      n, start, stop, final):
            # one DoubleRow matmul; final -> inc spe
            lstep = max(16, lhs_m)
            def mk():
                lv = WF8[0:128, lhs_col: lhs_col + lstep + lhs_m]
                lv = dataclasses.replace(lv, ap=[lv.ap[0], [lstep, 2], [1, lhs_m]])
                rv = rhs_buf[0:128, rhs_off: rhs_off + rhs_delta + n]
                rv = dataclasses.replace(rv, ap=[rv.ap[0], [rhs_delta, 2], [1, n]])
                ov = PS[0:lhs_m, psoff: psoff + n]
                return nc.tensor.matmul(ov, lv, rv, start=start, stop=stop,
                                        perf_mode=PM.DoubleRow)
            i = emit('pe', mk)
            if final:
                cnt['pe'] += 1
                inc(i, 'spe', 1)
            return i

        def evac(eng, PS, nchunks, nlast, dstbuf, dstoff, dst_ms, mpart,
                 sdma_guard):
            # evacuate psum chunks [512-strided, 508(or nlast) wide] -> dst
            # contiguous; sign/clip. eng in ('act','dve').
            wait(eng, 'spe', cnt['pe'])
            if dst_ms is not None:
                wait(eng, 'sgp', dst_ms)
            if sdma_guard is not None:
                wait(eng, sdma_guard[0], sdma_guard[1])
            total = 508 * (nchunks - 1) + nlast
            def mk():
                sv = PS[0:mpart, 0:(nchunks - 1) * 512 + nlast]
                sv = dataclasses.replace(sv, ap=[sv.ap[0], [512, nchunks], [1, 508]]) \
                    if nchunks > 1 else dataclasses.replace(sv, ap=[sv.ap[0], [1, nlast]])
                dv = dstbuf[0:mpart, dstoff: dstoff + total]
                if eng == 'act':
                    return nc.scalar.activation(dv, sv, AF.Sign)
                return nc.vector.tensor_scalar(dv, sv, 1.0, -1.0, ALU.min, ALU.max)
            i = emit(eng, mk)
            key = 'sact' if eng == 'act' else 'sdve'
            cnt[eng] += 1
            inc(i, key, 1)
            return (key, cnt[eng])

        dma_in_done = [None] * NIMG
        in_cnt = [0, 0]

        def emit_in_dma(j):
            # input DMA for image j (slot j%2); issued one image ahead.
            # Dedicated per-slot semaphore so the wait value is race-free
            # (only this slot's DMAs are ever outstanding on it).
            aslot = j % 2
            sname = f'sin{aslot}'
            off = aslot * 2560
            wait('sp', 'sgp', 2)      # A0 tail memsets done
            if a0_free[aslot] is not None:
                for sn, v in a0_free[aslot]:
                    wait('sp', sn, v)
            src_main = dataclasses.replace(
                x[j], ap=[[2048, 31], [65536, 3], [256, 10], [1, 256]])
            i = emit('sp', lambda src_main=src_main, off=off:
                     nc.sync.dma_start(A0[0:93, off:off + 2560], src_main))
            in_cnt[aslot] += 1
            inc(i, sname, 16)
            i = emit('sp', lambda j=j, off=off:
                     nc.sync.dma_start(A0[93:96, off:off + 2048], x[j, :, 248:256, :]))
            in_cnt[aslot] += 1
            inc(i, sname, 16)
            dma_in_done[j] = (sname, in_cnt[aslot])

        decomp_done = [None] * NIMG
        a0_read = [None] * NIMG

        def _decomp_chain(eng, sname, j, c0, c1):
            # one decomposition chain over A0-slot cols [c0, c1) on engine eng
            sl = j % 2
            aoff = sl * 2560
            hoff = sl * 7680
            E = {'gp': nc.gpsimd, 'dve': nc.vector}[eng]
            cw = c1 - c0
            wait(eng, dma_in_done[j][0], 16 * dma_in_done[j][1])
            if h_free[sl] is not None:
                wait(eng, 'spe', h_free[sl])
            i = emit(eng, lambda: E.tensor_copy(
                H[0:96, hoff + c0: hoff + c1], A0[0:96, aoff + c0: aoff + c1]))
            cnt[eng] += 1
            inc(i, sname, 1)
            wait(eng, sname, cnt[eng])
            i = emit(eng, lambda: E.tensor_tensor(
                R1[0:96, c0:c1], A0[0:96, aoff + c0: aoff + c1],
                H[0:96, hoff + c0: hoff + c1], ALU.subtract))
            cnt[eng] += 1
            inc(i, sname, 1)
            sub1 = (sname, cnt[eng])
            wait(eng, sname, cnt[eng])
            i = emit(eng, lambda: E.tensor_copy(
                H[0:96, hoff + 2560 + c0: hoff + 2560 + c1], R1[0:96, c0:c1]))
            cnt[eng] += 1
            inc(i, sname, 1)
            wait(eng, sname, cnt[eng])
            i = emit(eng, lambda: E.tensor_tensor(
                R2[0:96, c0:c1], R1[0:96, c0:c1],
                H[0:96, hoff + 2560 + c0: hoff + 2560 + c1], ALU.subtract))
            cnt[eng] += 1
            inc(i, sname, 1)
            wait(eng, sname, cnt[eng])
            i = emit(eng, lambda: E.tensor_copy(
                H[0:96, hoff + 5120 + c0: hoff + 5120 + c1], R2[0:96, c0:c1]))
            cnt[eng] += 1
            inc(i, sname, 1)
            return sub1, (sname, cnt[eng])

        def emit_decomp(j):
            # split fp32 A0 slot into 3 bf16 terms in H (exact to ~2^-25).
            # img0: two parallel half-chains (Pool + DVE) to shorten startup;
            # later images: single Pool chain overlapped with prior compute.
            sl = j % 2
            if j == 0:
                s1a, d1 = _decomp_chain('gp', 'sgp', j, 0, 1280)
                s1b, d2 = _decomp_chain('dve', 'sdve', j, 1280, 2560)
                a0_free[sl] = [s1a, s1b]
                decomp_done[j] = [d1, d2]
            else:
                if j == 1:
                    # img0's DVE half-chain shares R1/R2 scratch
                    wait('gp', decomp_done[0][1][0], decomp_done[0][1][1])
                s1, d1 = _decomp_chain('gp', 'sgp', j, 0, 2560)
                a0_free[sl] = [s1]
                decomp_done[j] = [d1]

        for img in range(NIMG):
            # ======== input DMA prefetch (this img on img0, next img after) ====
            if img == 0:
                emit_in_dma(0)
            if img + 1 < NIMG:
                emit_in_dma(img + 1)
            if img == 0:
                emit_decomp(0)
                # WF8 weight DMA after the first input DMAs
                i = emit('sp', lambda: nc.sync.dma_start(WF8[0:128, :], wf8[:]))
                inc(i, 'swf', 16)
            dma_in = dma_in_done[img]
            aslot = img % 2
            off = aslot * 2560

            # ======== L0: fp32 conv, 32 bands x 8 rows ========
            slot = tile_g % 2; tile_g += 1
            PS = P[slot]
            pe_tile_begin(slot, waits_other=list(decomp_done[img]), layer=0)
            hbase = aslot * 7680
            l0_chunk_pe = []
            for c in range(4):
                for t in range(3):
                    for tap in range(9):
                        ki, kj = tap // 3, tap % 3
                        rbase = hbase + t * 2560 + (2 * c + ki) * 256 + kj
                        def mk(c=c, rbase=rbase, tap=tap, PS=PS):
                            lhsT = WT0[0:96, tap * 128: tap * 128 + 128]
                            rv = H[0:96, rbase: rbase + 256 + 254]
                            rv = dataclasses.replace(rv,
                                                     ap=[rv.ap[0], [256, 2], [1, 254]])
                            ov = PS[0:128, c * 512: c * 512 + 508]
                            return nc.tensor.matmul(
                                ov, lhsT, rv, start=(t == 0 and tap == 0),
                                stop=(t == 2 and tap == 8))
                        i = emit('pe', mk)
                        if t == 2 and tap == 8:
                            cnt['pe'] += 1
                            inc(i, 'spe', 1)
                l0_chunk_pe.append(cnt['pe'])
            h_free[aslot] = cnt['pe']
            # prefetch: decompose the NEXT image during this image's layers
            if img + 1 < NIMG:
                emit_decomp(img + 1)

            # per-chunk pool pipeline: sign(c) -> vmax(c) -> hmax(c)
            vmax_cur = []
            for c in range(4):
                wait('act', 'spe', l0_chunk_pe[c])
                if vmax_prev is not None:
                    wait('act', 'sdve', vmax_prev[c])
                def mksgn(PS=PS, c=c):
                    return nc.scalar.activation(
                        T0B[0:128, c * 508: c * 508 + 508],
                        PS[0:128, c * 512: c * 512 + 508], AF.Sign)
                i = emit('act', mksgn)
                cnt['act'] += 1
                inc(i, 'sact', 1)
                if c == 3:
                    slot_free[slot] = ('sact', cnt['act'])
                # DVE vmax(c): rows 2c,2c+1 -> T1B row c
                wait('dve', 'sact', cnt['act'])
                def mkv(c=c):
                    a = T0B[0:128, c * 508: c * 508 + 254]
                    b = T0B[0:128, c * 508 + 254: c * 508 + 508]
                    return nc.vector.tensor_max(
                        T1B[0:128, c * 254: c * 254 + 254], a, b)
                i = emit('dve', mkv)
                cnt['dve'] += 1
                inc(i, 'sdve', 1)
                vmax_cur.append(cnt['dve'])
                # DVE hmax(c) -> A1 row c
                wait('dve', 'sdve', cnt['dve'])   # RAW on T1B
                if c == 0:
                    wait('dve', 'sgp', MS_A[1])
                    wait('dve', 'sh1', hcnt[1])
                def mkh(c=c):
                    sv = T1B[0:128, c * 254: c * 254 + 254]
                    a = dataclasses.replace(sv, ap=[sv.ap[0], [2, 127]])
                    b = dataclasses.replace(sv, offset=sv.offset + 1,
                                            ap=[sv.ap[0], [2, 127]])
                    return nc.vector.tensor_max(
                        A[1][0:128, c * WP: c * WP + 127], a, b)
                i = emit('dve', mkh)
                cnt['dve'] += 1
                inc(i, 'sdve', 1)
                if c == 1:
                    hmax01 = cnt['dve']
            hmax_all = cnt['dve']
            vmax_prev = vmax_cur

            # halo A1 (SP): bands g+1 rows 0:2 -> band g rows 4:6
            wait('sp', 'sdve', hmax01)
            i = emit('sp', lambda: nc.sync.dma_start(
                A[1][0:124, 4 * WP: 6 * WP], A[1][4:128, 0:2 * WP]))
            hcnt[1] += 16
            inc(i, 'sh1', 16)
            halo1 = ('sh1', hcnt[1])

            # ======== L1: G32 -> 16 out-bands, 2 phase-chunks ========
            slot = tile_g % 2; tile_g += 1
            PS = P[slot]
            pe_tile_begin(slot, waits_other=[halo1, ('sdve', hmax_all)])
            for p, (tA, tB) in enumerate(PAIRS):
                for ph in range(2):
                    kiA, kjA = tA
                    delta = 128 if tB else 16
                    dr_matmul(PS, ph * 512, OFF[1] + (p * 2 + ph) * 256, 128,
                              A[1], kiA * WP + kjA, delta, 508,
                              start=(p == 0), stop=(p == 5),
                              final=(p == 5 and ph == 1))
            # evac (DVE) -> A2 rows 0..8
            efree = evac('dve', PS, 2, 508, A[2], 0, MS_A[2], 128,
                         ('sh2', hcnt[2]) if hcnt[2] else None)
            l1_evac = efree
            slot_free[slot] = efree
            # halo A2
            wait('sp', efree[0], efree[1])
            i = emit('sp', lambda: nc.sync.dma_start(
                A[2][0:120, 8 * WP: 10 * WP], A[2][8:128, 0:2 * WP]))
            hcnt[2] += 16
            inc(i, 'sh2', 16)
            halo2 = ('sh2', hcnt[2])

            # ======== L2: G16 -> 8, chunks (ph, j) ========
            slot = tile_g % 2; tile_g += 1
            PS = P[slot]
            pe_tile_begin(slot, waits_other=[l1_evac])
            for j in range(2):
                if j == 1:
                    wait('pe', halo2[0], halo2[1])
                for ph in range(2):
                    for p, (tA, tB) in enumerate(PAIRS):
                        kiA, kjA = tA
                        delta = 128 if tB else 16
                        dr_matmul(PS, (2 * ph + j) * 512,
                                  OFF[2] + (p * 2 + ph) * 256, 128,
                                  A[2], (4 * j + kiA) * WP + kjA, delta, 508,
                                  start=(p == 0), stop=(p == 5),
                                  final=(p == 5 and ph == 1 and j == 1))
            # split evac: rows 0..8 first so L3's first chunk can start early
            eh1 = evac('act', PS, 2, 508, A[3], 0, MS_A[3], 128,
                       ('sh3', hcnt[3]) if hcnt[3] else None)
            l2_evac_h1 = eh1
            def mk_l2e2(PS=PS):
                sv = PS[0:128, 2 * 512: 3 * 512 + 508]
                sv = dataclasses.replace(sv, ap=[sv.ap[0], [512, 2], [1, 508]])
                dv = A[3][0:128, 8 * WP: 16 * WP]
                return nc.scalar.activation(dv, sv, AF.Sign)
            i = emit('act', mk_l2e2)
            cnt['act'] += 1
            inc(i, 'sact', 1)
            efree = ('sact', cnt['act'])
            l2_evac = efree
            slot_free[slot] = efree
            wait('sp', efree[0], efree[1])
            i = emit('sp', lambda: nc.sync.dma_start(
                A[3][0:112, 16 * WP: 18 * WP], A[3][16:128, 0:2 * WP]))
            hcnt[3] += 16
            inc(i, 'sh3', 16)
            halo3 = ('sh3', hcnt[3])

            # ======== L3: G8 -> 4, two row-tiles (tau = phase) ========
            l3_evacs = []
            for tau in range(2):
                slot = tile_g % 2; tile_g += 1
                PS = P[slot]
                pe_tile_begin(slot, waits_other=[l2_evac_h1])
                for j in range(4):
                    if j == 1:
                        wait('pe', l2_evac[0], l2_evac[1])
                    if j == 3:
                        wait('pe', halo3[0], halo3[1])
                    for p, (tA, tB) in enumerate(PAIRS):
                        kiA, kjA = tA
                        delta = 128 if tB else 16
                        dr_matmul(PS, j * 512, OFF[3] + (p * 2 + tau) * 256, 128,
                                  A[3], (4 * j + kiA) * WP + kjA, delta, 508,
                                  start=(p == 0), stop=(p == 5),
                                  final=(p == 5 and j == 3))
                efree = evac('dve', PS, 4, 508, A[4], tau * 16 * WP, MS_A[4], 128,
                             ('sh4', hcnt[4]) if (tau == 0 and hcnt[4]) else None)
                slot_free[slot] = efree
                l3_evacs.append(efree)
                if tau == 0:
                    wait('sp', efree[0], efree[1])
                    i = emit('sp', lambda: nc.sync.dma_start(
                        A[4][0:96, 32 * WP: 34 * WP], A[4][32:128, 0:2 * WP]))
                    hcnt[4] += 16
                    inc(i, 'sh4', 16)
                    halo4 = ('sh4', hcnt[4])

            # ======== L4: G4, tiles (h, tau) ========
            l4_t0_evacs = []
            l4_evac_last = None
            for h in range(2):
                for tau in range(2):
                    slot = tile_g % 2; tile_g += 1
                    PS = P[slot]
                    pe_tile_begin(slot, waits_other=[l3_evacs[tau]])
                    for j in range(4):
                        if j == 3:
                            if tau == 0:
                                wait('pe', l3_evacs[1][0], l3_evacs[1][1])
                            else:
                                wait('pe', halo4[0], halo4[1])
                        for p, (tA, tB) in enumerate(PAIRS):
                            kiA, kjA = tA
                            delta = 128 if tB else 16
                            dr_matmul(PS, j * 512, OFF[4] + (p * 2 + h) * 256, 128,
                                      A[4], (16 * tau + 4 * j + kiA) * WP + kjA,
                                      delta, 508,
                                      start=(p == 0), stop=(p == 5),
                                      final=(p == 5 and j == 3))
                    efree = evac('act', PS, 4, 508, A[5],
                                 h * SL5 + tau * 16 * WP, MS_A[5], 128,
                                 ('sh5', hcnt[5]) if (h == 0 and tau == 0 and hcnt[5]) else None)
                    slot_free[slot] = efree
                    l4_evac_last = efree
                    if tau == 0:
                        l4_t0_evacs.append(efree)
            # halo A5 (both slabs, 1 DMA) after (h0,t0) and (h1,t0) evacs
            wait('sp', l4_t0_evacs[1][0], l4_t0_evacs[1][1])
            def mkh5():
                sv = A[5][32:128, 0:SL5 + 2 * WP]
                sv = dataclasses.replace(sv, ap=[sv.ap[0], [SL5, 2], [1, 2 * WP]])
                dv = A[5][0:96, 32 * WP: SL5 + 34 * WP]
                dv = dataclasses.replace(dv, ap=[dv.ap[0], [SL5, 2], [1, 2 * WP]])
                return nc.sync.dma_start(dv, sv)
            i = emit('sp', mkh5)
            hcnt[5] += 16
            inc(i, 'sh5', 16)
            halo5 = ('sh5', hcnt[5])

            # ======== L5: G4, t1 (k-split slabs), 9 taps, two row-tiles ========
            l5_evacs = []
            for tau in range(2):
                slot = tile_g % 2; tile_g += 1
                PS = P[slot]
                if tau == 0:
                    pe_tile_begin(slot, waits_other=[l4_t0_evacs[1]])
                else:
                    pe_tile_begin(slot, waits_other=[l4_evac_last])
                for j in range(4):
                    if j == 3:
                        if tau == 0:
                            wait('pe', l4_evac_last[0], l4_evac_last[1])
                        else:
                            wait('pe', halo5[0], halo5[1])
                    for tap in range(9):
                        ki, kj = tap // 3, tap % 3
                        dr_matmul(PS, j * 512, OFF[5] + tap * 256, 128,
                                  A[5], (16 * tau + 4 * j + ki) * WP + kj,
                                  SL5, 508,
                                  start=(tap == 0), stop=(tap == 8),
                                  final=(tap == 8 and j == 3))
                efree = evac('dve', PS, 4, 508, A[6], tau * 16 * WP, MS_A[6], 128,
                             ('sh6', hcnt[6]) if (tau == 0 and hcnt[6]) else None)
                slot_free[slot] = efree
                l5_evacs.append(efree)
                if tau == 0:
                    wait('sp', efree[0], efree[1])
                    i = emit('sp', lambda: nc.sync.dma_start(
                        A[6][0:96, 32 * WP: 34 * WP], A[6][32:128, 0:2 * WP]))
                    hcnt[6] += 16
                    inc(i, 'sh6', 16)
                    halo6 = ('sh6', hcnt[6])

            # ======== L6: G4, M=8, two row-tiles ========
            l6_evacs = []
            for tau in range(2):
                slot = tile_g % 2; tile_g += 1
                PS = P[slot]
                pe_tile_begin(slot, waits_other=[l5_evacs[tau]])
                for j in range(4):
                    if j == 3:
                        if tau == 0:
                            wait('pe', l5_evacs[1][0], l5_evacs[1][1])
                        else:
                            wait('pe', halo6[0], halo6[1])
                    for p, (tA, tB) in enumerate(PAIRS):
                        kiA, kjA = tA
                        delta = 128 if tB else 16
                        dr_matmul(PS, j * 512, OFF[6] + p * 32, 8,
                                  A[6], (16 * tau + 4 * j + kiA) * WP + kjA,
                                  delta, 508,
                                  start=(p == 0), stop=(p == 5),
                                  final=(p == 5 and j == 3))
                wait('act', 'spe', cnt['pe'])
                if tau == 0 and out_cnt:
                    wait('act', 'sout', out_cnt)
                def mk6(PS=PS, tau=tau):
                    sv = PS[0:8, 0:3 * 512 + 508]
                    sv = dataclasses.replace(
                        sv, ap=[sv.ap[0], [512, 4], [127, 4], [1, 115]])
                    dv = OUTB[0:8, tau * 16 * 115: tau * 16 * 115 + 16 * 115]
                    return nc.scalar.activation(dv, sv, AF.Sign)
                i = emit('act', mk6)
                cnt['act'] += 1
                inc(i, 'sact', 1)
                efree = ('sact', cnt['act'])
                slot_free[slot] = efree
                l6_evacs.append(efree)

            # ======== output DMA: 8 flat per-(band,ch) planes ========
            wait('sp', l6_evacs[1][0], l6_evacs[1][1])
            for g in range(4):
                nrows = 32 if g < 3 else 19
                for c in range(2):
                    def mko(img=img, g=g, c=c, nrows=nrows):
                        sv = OUTB[2 * g + c: 2 * g + c + 1, 0: nrows * 115]
                        dv = y[img, 0:1]
                        dv = dataclasses.replace(
                            dv, offset=dv.offset + c * 13225 + 32 * g * 115,
                            ap=[[1, nrows * 115]])
                        return nc.sync.dma_start(dv, sv)
                    i = emit('sp', mko)
                    out_cnt += 16
                    inc(i, 'sout', 16)
        return cnt

    with nc.Block() as block:
        @block.tensor
        def _(E):
            walk(E, 'pe')

        @block.scalar
        def _(E):
            walk(E, 'act')

        @block.vector
        def _(E):
            walk(E, 'dve')

        @block.gpsimd
        def _(E):
            walk(E, 'gp')

        @block.sync
        def _(E):
            walk(E, 'sp')

    for cm in reversed(ctxs):
        cm.__exit__(None, None, None)
    return nc


def pack_weights(ws):
    """ws: 7 raw arrays (cout, cin, 3, 3) -> (w0f fp32, wf8 fp8)."""
    import ml_dtypes
    sws = [np.sign(w).astype(np.float32) for w in ws]
    # L0: 32 bands x 3cin on 96 partitions -> 128 out (32 bands x 4)
    w0f = np.zeros((96, 9 * 128), np.float32)
    for tap in range(9):
        ki, kj = tap // 3, tap % 3
        blk = sws[0][:, :, ki, kj].T  # (cin, cout)
        for s in range(32):
            w0f[s * 3:s * 3 + 3, tap * 128 + s * 4: tap * 128 + s * 4 + 4] = blk
    w0f = w0f.astype(ml_dtypes.bfloat16)
    wf8 = np.zeros((128, WF8_COLS), np.float32)
    # t2 layers: 1,2,3 (phases), 4 (cout halves), 6 (plain)
    for l, nph in ((1, 2), (2, 2), (3, 2)):
        cin, cout = CH[l]
        gin = G[l]
        M = 128
        for p, (tA, tB) in enumerate(PAIRS):
            for ph in range(nph):
                col = OFF[l] + (p * 2 + ph) * 256
                for i, tap in enumerate((tA, tB)):
                    if tap is None:
                        continue
                    ki, kj = tap
                    blk = sws[l][:, :, ki, kj].T  # (cin, cout)
                    for gp_ in range(gin // 2):
                        g = 2 * gp_ + ph
                        wf8[g * cin:(g + 1) * cin,
                            col + i * M + gp_ * cout: col + i * M + (gp_ + 1) * cout] = blk
    # L4: cout halves
    cin, cout = CH[4]
    for p, (tA, tB) in enumerate(PAIRS):
        for h in range(2):
            col = OFF[4] + (p * 2 + h) * 256
            for i, tap in enumerate((tA, tB)):
                if tap is None:
                    continue
                ki, kj = tap
                blk = sws[4][32 * h:32 * h + 32, :, ki, kj].T  # (32cin, 32cout)
                for g in range(4):
                    wf8[g * 32:(g + 1) * 32,
                        col + i * 128 + g * 32: col + i * 128 + (g + 1) * 32] = blk
    # L5: t1 k-split (slab i = channels 32i..32i+32)
    for tap in range(9):
        ki, kj = tap // 3, tap % 3
        col = OFF[5] + tap * 256
        for i in range(2):
            blk = sws[5][:, 32 * i:32 * i + 32, ki, kj].T  # (32cin-half, 32cout)
            for g in range(4):
                wf8[g * 32:(g + 1) * 32,
                    col + i * 128 + g * 32: col + i * 128 + (g + 1) * 32] = blk
    # L6: M=8 (ktile step padded to 16)
    for p, (tA, tB) in enumerate(PAIRS):
        col = OFF[6] + p * 32
        for i, tap in enumerate((tA, tB)):
            if tap is None:
                continue
            ki, kj = tap
            blk = sws[6][:, :, ki, kj].T  # (32, 2)
            for g in range(4):
                wf8[g * 32:(g + 1) * 32,
                    col + i * 16 + g * 2: col + i * 16 + (g + 1) * 2] = blk
    return w0f, wf8.astype(ml_dtypes.float8_e4m3fn)


LAST_RESULTS = None


def kernel(**inputs):
    global LAST_RESULTS
    from concourse.bass_utils import run_bass_kernel_spmd
    inp = np.asarray(inputs['inputs'], np.float32)
    ws = [np.asarray(inputs[f'w{i}']) for i in range(7)]
    w0f, wf8 = pack_weights(ws)
    nc = build_program()
    in_maps = []
    for c in range(8):
        in_maps.append({'x': np.ascontiguousarray(inp[c * 8:(c + 1) * 8]),
                        'w0f': w0f, 'wf8': wf8})
    res = run_bass_kernel_spmd(nc, in_maps, core_ids=list(range(8)),
                               tmpdir=os.environ.get('KERNEL_TRACE_DIR') or None)
    LAST_RESULTS = res
    out = np.concatenate([res.results[c]['y'] for c in range(8)], axis=0)
    return out.astype(np.float32)



# revision 4
# speedup vs baseline: 1.0474x; 1.0474x over previous
"""Trainium2 Bass kernel v3 for the 7-layer binarized CNN (nn_MCNET).

Data parallel over 8 cores (8 images each). Per core:
- L0 (3->4, fp32 input): input split HOST-SIDE into two 11-bit-significand
  fp32 terms (t1 = round11(x), t2 = round11(x - t1), residual <= 2^-22|x|),
  fed to the PE as float32r (TRN2 PE keeps exactly 11 mantissa bits on the
  fp32r moving path, so both terms pass through exactly; verified on HW).
  18 accumulating fp32r matmuls per psum chunk (vs 27 bf16 in v2) over
  block-diagonal weights (32 row-bands x 3cin on 96 partitions -> 128 out).
  No on-device decomposition: Pool only does init memsets.
- Per-psum-chunk ACT Sign -> bf16, DVE 2x2 maxpool writes fp8 directly
  into A1's banded layout.
- L1..L6: fp8e4 DoubleRow matmuls. Activations live in per-layer banded
  buffers A_l: G bands x cin channels on 128 partitions, rows contiguous at
  stride 127 (127 % 16 == 15, so tap pairs (ki,kj)->(ki+1,kj+1) have ktile
  stride 128, a legal DoubleRow step). Tap-paired layers run 6 DoubleRow
  passes instead of 9; L5 (cin=64) k-splits channels across two 4320-byte
  slabs instead. Band nesting is chosen so every PSUM evacuation is an
  identity-partition ACT/DVE op (sign == clip for the even-integer sums),
  and inter-band halos are ONE contiguous partition-shifted SBUF DMA per
  layer. All DMAs issue from the SP (sync) engine -> HWDGE.
- Output stored/DMA'd as fp8e4 (values in {-1,0,1} exact), converted to
  fp32 on host.
"""
import sys, os, dataclasses
sys.path.insert(0, '/opt/trn_rl_repo')
import numpy as np

CH = [(3, 4), (4, 8), (8, 16), (16, 32), (32, 64), (64, 32), (32, 2)]
HIN = [256, 127, 125, 123, 121, 119, 117]
HOUT = [h - 2 for h in HIN]
G = [32, 32, 16, 8, 4, 4, 4]         # in-bands per layer
BB = [8, 4, 8, 16, 32, 32, 32]       # nominal in-band rows
WP = 127                              # fp8 row stride (127 % 16 == 15)
SL5 = 4320                            # A5 slab stride (34*127=4318 -> pad to %16)
NIMG = 8
# tap pairs for t2 (tap-paired DoubleRow): ktile delta = 127*dki + dkj = 128
PAIRS = [((0, 0), (1, 1)), ((0, 1), (1, 2)), ((1, 0), (2, 1)),
         ((0, 2), None), ((2, 0), None), ((2, 2), None)]
# weight block column offsets in WF8
NB = [0, 12, 12, 12, 12, 9, 6]        # lhsT blocks per layer (l1..l6 used)
OFF = {}
_c = 0
for _l in range(1, 7):
    OFF[_l] = _c
    _c += NB[_l] * (256 if _l < 6 else 32)
WF8_COLS = _c
A_ROWS = [0, 6, 10, 18, 34, 0, 34]    # stored rows per band (A5 special)
A_COLS = [0] + [A_ROWS[l] * WP + 384 for l in range(1, 7)]
A_COLS[5] = 2 * SL5 + 384
TSLOT = 5120                          # T cols per slot: 2 terms x 10 rows x 256


def build_program():
    import concourse.bass as bass
    import concourse.mybir as mybir
    dt = mybir.dt
    AF = mybir.ActivationFunctionType
    PM = mybir.MatmulPerfMode
    ALU = mybir.AluOpType

    nc = bass.Bass("TRN2", target_bir_lowering=False)
    x = nc.dram_tensor("x", (NIMG, 2, 3, 256, 256), dt.float32r,
                       kind="ExternalInput")
    w0f = nc.dram_tensor("w0f", (96, 9 * 128), dt.float32r, kind="ExternalInput")
    wf8 = nc.dram_tensor("wf8", (128, WF8_COLS), dt.float8e4, kind="ExternalInput")
    y = nc.dram_tensor("y", (NIMG, 2 * 115 * 115), dt.float8e4,
                       kind="ExternalOutput")

    ctxs = []
    def alloc(cm):
        ctxs.append(cm)
        return cm.__enter__()

    W0F = alloc(nc.sbuf_tensor("W0F", [96, 9 * 128], dt.float32r))
    WF8 = alloc(nc.sbuf_tensor("WF8", [128, WF8_COLS], dt.float8e4))
    T = alloc(nc.sbuf_tensor("T", [96, 2 * TSLOT], dt.float32r))
    A = [None] * 7
    for l in range(1, 7):
        A[l] = alloc(nc.sbuf_tensor(f"A{l}", [128, A_COLS[l]], dt.float8e4))
    T0B = alloc(nc.sbuf_tensor("T0B", [128, 2032], dt.bfloat16))
    T1B = alloc(nc.sbuf_tensor("T1B", [128, 4 * 254], dt.bfloat16))
    OUTB = alloc(nc.sbuf_tensor("OUTB", [128, 32 * 115], dt.float8e4))
    P = [alloc(nc.psum_tensor(f"P{i}", [128, 2048], dt.float32)) for i in range(2)]
    sem = {n: alloc(nc.semaphore(name=n)) for n in
           ['sdma', 'spe', 'sact', 'sdve', 'sgp', 'sin0', 'sin1', 'swf',
            'sh1', 'sh2', 'sh3', 'sh4', 'sh5', 'sh6', 'sout']}

    def walk(E, me):
        cnt = {'dma': 0, 'pe': 0, 'act': 0, 'dve': 0, 'gp': 0}
        last_wait = {}

        def wait(eng, semn, val):
            if val is None or val <= 0:
                return
            k = (eng, semn)
            if last_wait.get(k, -1) >= val:
                return
            last_wait[k] = val
            if eng == me:
                E.wait_ge(sem[semn], val)

        def emit(eng, fn):
            if eng == me:
                return fn()
            return None

        def inc(inst, semn, v):
            if inst is not None:
                inst.then_inc(sem[semn], v)

        # ---- init: memsets ----
        # T: band 31's rows 8,9 (cols 2048:2560 of each term block) are never
        # DMA-written; zero once per (slot, term).
        for sl in range(2):
            for t in range(2):
                i = emit('gp', lambda sl=sl, t=t: nc.gpsimd.memset(
                    T[64:96, sl * TSLOT + t * 2560 + 2048:
                      sl * TSLOT + t * 2560 + 2560], 0.0))
                cnt['gp'] += 1
                inc(i, 'sgp', 1)
        for l in range(1, 7):
            i = emit('gp', lambda l=l:
                     nc.gpsimd.memset(A[l][0:128, 0:A_COLS[l]], 0.0))
            cnt['gp'] += 1
            inc(i, 'sgp', 1)
        MS_T = 4
        MS_A = {1: 5, 2: 6, 3: 7, 4: 8, 5: 9, 6: 10}  # sgp count when A_l ready
        # ---- weight DMA: W0F first (L0); WF8 after img0 input ----
        i = emit('sp', lambda: nc.sync.dma_start(W0F[0:96, :], w0f[:]))
        inc(i, 'sdma', 16)

        slot_free = [None, None]
        t_free = [None, None]        # spe count when T slot fully read
        vmax_prev = None            # sdve count of prev img vmax (T0B free)
        hcnt = {l: 0 for l in range(1, 7)}   # per-halo-sem cumulative counts
        out_cnt = 0
        tile_g = 0

        def pe_tile_begin(slot, waits_other=(), layer=1):
            if layer == 0:
                wait('pe', 'sgp', MS_T)
                wait('pe', 'sdma', 16)      # W0F
            else:
                wait('pe', 'sgp', MS_A[layer])
                wait('pe', 'swf', 16)       # WF8
            for sname, v in waits_other:
                wait('pe', sname, v)
            if slot_free[slot] is not None:
                wait('pe', slot_free[slot][0], slot_free[slot][1])

        def dr_matmul(PS, psoff, lhs_col, lhs_m, rhs_buf, rhs_off, rhs_delta,
                      n, start, stop, final):
            lstep = max(16, lhs_m)
            def mk():
                lv = WF8[0:128, lhs_col: lhs_col + lstep + lhs_m]
                lv = dataclasses.replace(lv, ap=[lv.ap[0], [lstep, 2], [1, lhs_m]])
                rv = rhs_buf[0:128, rhs_off: rhs_off + rhs_delta + n]
                rv = dataclasses.replace(rv, ap=[rv.ap[0], [rhs_delta, 2], [1, n]])
                ov = PS[0:lhs_m, psoff: psoff + n]
                return nc.tensor.matmul(ov, lv, rv, start=start, stop=stop,
                                        perf_mode=PM.DoubleRow)
            i = emit('pe', mk)
            if final:
                cnt['pe'] += 1
                inc(i, 'spe', 1)
            return i

        def evac(eng, PS, nchunks, nlast, dstbuf, dstoff, dst_ms, mpart,
                 sdma_guard):
            wait(eng, 'spe', cnt['pe'])
            if dst_ms is not None:
                wait(eng, 'sgp', dst_ms)
            if sdma_guard is not None:
                wait(eng, sdma_guard[0], sdma_guard[1])
            total = 508 * (nchunks - 1) + nlast
            def mk():
                sv = PS[0:mpart, 0:(nchunks - 1) * 512 + nlast]
                sv = dataclasses.replace(sv, ap=[sv.ap[0], [512, nchunks], [1, 508]]) \
                    if nchunks > 1 else dataclasses.replace(sv, ap=[sv.ap[0], [1, nlast]])
                dv = dstbuf[0:mpart, dstoff: dstoff + total]
                if eng == 'act':
                    return nc.scalar.activation(dv, sv, AF.Sign)
                return nc.vector.tensor_scalar(dv, sv, 1.0, -1.0, ALU.min, ALU.max)
            i = emit(eng, mk)
            key = 'sact' if eng == 'act' else 'sdve'
            cnt[eng] += 1
            inc(i, key, 1)
            return (key, cnt[eng])

        dma_in_done = [None] * NIMG
        dma_in_t0 = [None] * NIMG
        in_cnt = [0, 0]

        def emit_in_dma(j):
            # input DMA for image j (slot j%2): per term, main (93 parts) +
            # tail (3 parts). Dedicated per-slot semaphore (race-free counts).
            aslot = j % 2
            sname = f'sin{aslot}'
            toff = aslot * TSLOT
            wait('sp', 'sgp', MS_T)
            if t_free[aslot] is not None:
                wait('sp', 'spe', t_free[aslot])
            for t in range(2):
                src_main = dataclasses.replace(
                    x[j, t], ap=[[2048, 31], [65536, 3], [256, 10], [1, 256]])
                i = emit('sp', lambda src_main=src_main, toff=toff, t=t:
                         nc.sync.dma_start(
                             T[0:93, toff + t * 2560: toff + t * 2560 + 2560],
                             src_main))
                in_cnt[aslot] += 1
                inc(i, sname, 16)
                i = emit('sp', lambda j=j, t=t, toff=toff:
                         nc.sync.dma_start(
                             T[93:96, toff + t * 2560: toff + t * 2560 + 2048],
                             x[j, t, :, 248:256, :]))
                in_cnt[aslot] += 1
                inc(i, sname, 16)
                if t == 0:
                    dma_in_t0[j] = (sname, in_cnt[aslot])
            dma_in_done[j] = (sname, in_cnt[aslot])

        for img in range(NIMG):
            # ======== input DMA prefetch (this img on img0, next img after) ====
            if img == 0:
                emit_in_dma(0)
            if img + 1 < NIMG:
                emit_in_dma(img + 1)
            if img == 0:
                # WF8 weight DMA after the first input DMAs
                i = emit('sp', lambda: nc.sync.dma_start(WF8[0:128, :], wf8[:]))
                inc(i, 'swf', 16)
            dma_in = dma_in_done[img]
            aslot = img % 2
            toff = aslot * TSLOT

            # ======== L0: fp32r conv, 32 bands x 8 rows, 2 host-split terms ====
            slot = tile_g % 2; tile_g += 1
            PS = P[slot]
            pe_tile_begin(slot, layer=0)
            l0_chunk_pe = [None] * 4

            def l0_mm(c, t, tap, PS, final):
                ki, kj = tap // 3, tap % 3
                rbase = toff + t * 2560 + (2 * c + ki) * 256 + kj
                def mk(c=c, rbase=rbase, tap=tap, PS=PS, t=t):
                    lhsT = W0F[0:96, tap * 128: tap * 128 + 128]
                    rv = T[0:96, rbase: rbase + 256 + 254]
                    rv = dataclasses.replace(rv,
                                             ap=[rv.ap[0], [256, 2], [1, 254]])
                    ov = PS[0:128, c * 512: c * 512 + 508]
                    return nc.tensor.matmul(
                        ov, lhsT, rv, start=(t == 0 and tap == 0),
                        stop=(t == 1 and tap == 8))
                i = emit('pe', mk)
                if final:
                    cnt['pe'] += 1
                    inc(i, 'spe', 1)

            if img == 0:
                # term-outer: PE starts after term-0 DMA only
                for t in range(2):
                    dm = dma_in_t0[0] if t == 0 else dma_in
                    wait('pe', dm[0], 16 * dm[1])
                    for c in range(4):
                        for tap in range(9):
                            final = (t == 1 and tap == 8)
                            l0_mm(c, t, tap, PS, final)
                            if final:
                                l0_chunk_pe[c] = cnt['pe']
            else:
                wait('pe', dma_in[0], 16 * dma_in[1])
                for c in range(4):
                    for t in range(2):
                        for tap in range(9):
                            final = (t == 1 and tap == 8)
                            l0_mm(c, t, tap, PS, final)
                            if final:
                                l0_chunk_pe[c] = cnt['pe']
            t_free[aslot] = cnt['pe']

            # per-chunk pool pipeline: sign(c) -> vmax(c) -> hmax(c)
            vmax_cur = []
            for c in range(4):
                wait('act', 'spe', l0_chunk_pe[c])
                if vmax_prev is not None:
                    wait('act', 'sdve', vmax_prev[c])
                def mksgn(PS=PS, c=c):
                    return nc.scalar.activation(
                        T0B[0:128, c * 508: c * 508 + 508],
                        PS[0:128, c * 512: c * 512 + 508], AF.Sign)
                i = emit('act', mksgn)
                cnt['act'] += 1
                inc(i, 'sact', 1)
                if c == 3:
                    slot_free[slot] = ('sact', cnt['act'])
                wait('dve', 'sact', cnt['act'])
                def mkv(c=c):
                    a = T0B[0:128, c * 508: c * 508 + 254]
                    b = T0B[0:128, c * 508 + 254: c * 508 + 508]
                    return nc.vector.tensor_max(
                        T1B[0:128, c * 254: c * 254 + 254], a, b)
                i = emit('dve', mkv)
                cnt['dve'] += 1
                inc(i, 'sdve', 1)
                vmax_cur.append(cnt['dve'])
                wait('dve', 'sdve', cnt['dve'])   # RAW on T1B
                if c == 0:
                    wait('dve', 'sgp', MS_A[1])
                    wait('dve', 'sh1', hcnt[1])
                def mkh(c=c):
                    sv = T1B[0:128, c * 254: c * 254 + 254]
                    a = dataclasses.replace(sv, ap=[sv.ap[0], [2, 127]])
                    b = dataclasses.replace(sv, offset=sv.offset + 1,
                                            ap=[sv.ap[0], [2, 127]])
                    return nc.vector.tensor_max(
                        A[1][0:128, c * WP: c * WP + 127], a, b)
                i = emit('dve', mkh)
                cnt['dve'] += 1
                inc(i, 'sdve', 1)
                if c == 1:
                    hmax01 = cnt['dve']
            hmax_all = cnt['dve']
            vmax_prev = vmax_cur

            # halo A1 (SP): bands g+1 rows 0:2 -> band g rows 4:6
            wait('sp', 'sdve', hmax01)
            i = emit('sp', lambda: nc.sync.dma_start(
                A[1][0:124, 4 * WP: 6 * WP], A[1][4:128, 0:2 * WP]))
            hcnt[1] += 16
            inc(i, 'sh1', 16)
            halo1 = ('sh1', hcnt[1])

            # ======== L1: G32 -> 16 out-bands, 2 phase-chunks ========
            slot = tile_g % 2; tile_g += 1
            PS = P[slot]
            pe_tile_begin(slot, waits_other=[halo1, ('sdve', hmax_all)])
            for p, (tA, tB) in enumerate(PAIRS):
                for ph in range(2):
                    kiA, kjA = tA
                    delta = 128 if tB else 16
                    dr_matmul(PS, ph * 512, OFF[1] + (p * 2 + ph) * 256, 128,
                              A[1], kiA * WP + kjA, delta, 508,
                              start=(p == 0), stop=(p == 5),
                              final=(p == 5 and ph == 1))
            efree = evac('dve', PS, 2, 508, A[2], 0, MS_A[2], 128,
                         ('sh2', hcnt[2]) if hcnt[2] else None)
            l1_evac = efree
            slot_free[slot] = efree
            wait('sp', efree[0], efree[1])
            i = emit('sp', lambda: nc.sync.dma_start(
                A[2][0:120, 8 * WP: 10 * WP], A[2][8:128, 0:2 * WP]))
            hcnt[2] += 16
            inc(i, 'sh2', 16)
            halo2 = ('sh2', hcnt[2])

            # ======== L2: G16 -> 8, chunks (ph, j) ========
            slot = tile_g % 2; tile_g += 1
            PS = P[slot]
            pe_tile_begin(slot, waits_other=[l1_evac])
            for j in range(2):
                if j == 1:
                    wait('pe', halo2[0], halo2[1])
                for ph in range(2):
                    for p, (tA, tB) in enumerate(PAIRS):
                        kiA, kjA = tA
                        delta = 128 if tB else 16
                        dr_matmul(PS, (2 * ph + j) * 512,
                                  OFF[2] + (p * 2 + ph) * 256, 128,
                                  A[2], (4 * j + kiA) * WP + kjA, delta, 508,
                                  start=(p == 0), stop=(p == 5),
                                  final=(p == 5 and ph == 1 and j == 1))
            eh1 = evac('act', PS, 2, 508, A[3], 0, MS_A[3], 128,
                       ('sh3', hcnt[3]) if hcnt[3] else None)
            l2_evac_h1 = eh1
            def mk_l2e2(PS=PS):
                sv = PS[0:128, 2 * 512: 3 * 512 + 508]
                sv = dataclasses.replace(sv, ap=[sv.ap[0], [512, 2], [1, 508]])
                dv = A[3][0:128, 8 * WP: 16 * WP]
                return nc.scalar.activation(dv, sv, AF.Sign)
            i = emit('act', mk_l2e2)
            cnt['act'] += 1
            inc(i, 'sact', 1)
            efree = ('sact', cnt['act'])
            l2_evac = efree
            slot_free[slot] = efree
            # halo3 source rows 0,1 are in the h1 evac -> issue early
            wait('sp', l2_evac_h1[0], l2_evac_h1[1])
            i = emit('sp', lambda: nc.sync.dma_start(
                A[3][0:112, 16 * WP: 18 * WP], A[3][16:128, 0:2 * WP]))
            hcnt[3] += 16
            inc(i, 'sh3', 16)
            halo3 = ('sh3', hcnt[3])

            # ======== L3: G8 -> 4, two row-tiles (tau = phase) ========
            l3_evacs = []
            for tau in range(2):
                slot = tile_g % 2; tile_g += 1
                PS = P[slot]
                pe_tile_begin(slot, waits_other=[l2_evac_h1])
                for j in range(4):
                    if j == 1:
                        wait('pe', l2_evac[0], l2_evac[1])
                    if j == 3:
                        wait('pe', halo3[0], halo3[1])
                    for p, (tA, tB) in enumerate(PAIRS):
                        kiA, kjA = tA
                        delta = 128 if tB else 16
                        dr_matmul(PS, j * 512, OFF[3] + (p * 2 + tau) * 256, 128,
                                  A[3], (4 * j + kiA) * WP + kjA, delta, 508,
                                  start=(p == 0), stop=(p == 5),
                                  final=(p == 5 and j == 3))
                efree = evac('dve', PS, 4, 508, A[4], tau * 16 * WP, MS_A[4], 128,
                             ('sh4', hcnt[4]) if (tau == 0 and hcnt[4]) else None)
                slot_free[slot] = efree
                l3_evacs.append(efree)
                if tau == 0:
                    wait('sp', efree[0], efree[1])
                    i = emit('sp', lambda: nc.sync.dma_start(
                        A[4][0:96, 32 * WP: 34 * WP], A[4][32:128, 0:2 * WP]))
                    hcnt[4] += 16
                    inc(i, 'sh4', 16)
                    halo4 = ('sh4', hcnt[4])

            # ======== L4: G4, tiles (h, tau) ========
            l4_t0_evacs = []
            l4_evac_last = None
            for h in range(2):
                for tau in range(2):
                    slot = tile_g % 2; tile_g += 1
                    PS = P[slot]
                    pe_tile_begin(slot, waits_other=[l3_evacs[tau]])
                    for j in range(4):
                        if j == 3:
                            if tau == 0:
                                wait('pe', l3_evacs[1][0], l3_evacs[1][1])
                            else:
                                wait('pe', halo4[0], halo4[1])
                        for p, (tA, tB) in enumerate(PAIRS):
                            kiA, kjA = tA
                            delta = 128 if tB else 16
                            dr_matmul(PS, j * 512, OFF[4] + (p * 2 + h) * 256, 128,
                                      A[4], (16 * tau + 4 * j + kiA) * WP + kjA,
                                      delta, 508,
                                      start=(p == 0), stop=(p == 5),
                                      final=(p == 5 and j == 3))
                    efree = evac('act', PS, 4, 508, A[5],
                                 h * SL5 + tau * 16 * WP, MS_A[5], 128,
                                 ('sh5', hcnt[5]) if (h == 0 and tau == 0 and hcnt[5]) else None)
                    slot_free[slot] = efree
                    l4_evac_last = efree
                    if tau == 0:
                        l4_t0_evacs.append(efree)
            wait('sp', l4_t0_evacs[1][0], l4_t0_evacs[1][1])
            def mkh5():
                sv = A[5][32:128, 0:SL5 + 2 * WP]
                sv = dataclasses.replace(sv, ap=[sv.ap[0], [SL5, 2], [1, 2 * WP]])
                dv = A[5][0:96, 32 * WP: SL5 + 34 * WP]
                dv = dataclasses.replace(dv, ap=[dv.ap[0], [SL5, 2], [1, 2 * WP]])
                return nc.sync.dma_start(dv, sv)
            i = emit('sp', mkh5)
            hcnt[5] += 16
            inc(i, 'sh5', 16)
            halo5 = ('sh5', hcnt[5])

            # ======== L5: G4, t1 (k-split slabs), 9 taps, two row-tiles ========
            l5_evacs = []
            for tau in range(2):
                slot = tile_g % 2; tile_g += 1
                PS = P[slot]
                if tau == 0:
                    pe_tile_begin(slot, waits_other=[l4_t0_evacs[1]])
                else:
                    pe_tile_begin(slot, waits_other=[l4_evac_last])
                for j in range(4):
                    if j == 3:
                        if tau == 0:
                            wait('pe', l4_evac_last[0], l4_evac_last[1])
                        else:
                            wait('pe', halo5[0], halo5[1])
                    for tap in range(9):
                        ki, kj = tap // 3, tap % 3
                        dr_matmul(PS, j * 512, OFF[5] + tap * 256, 128,
                                  A[5], (16 * tau + 4 * j + ki) * WP + kj,
                                  SL5, 508,
                                  start=(tap == 0), stop=(tap == 8),
                                  final=(tap == 8 and j == 3))
                efree = evac('dve', PS, 4, 508, A[6], tau * 16 * WP, MS_A[6], 128,
                             ('sh6', hcnt[6]) if (tau == 0 and hcnt[6]) else None)
                slot_free[slot] = efree
                l5_evacs.append(efree)
                if tau == 0:
                    wait('sp', efree[0], efree[1])
                    i = emit('sp', lambda: nc.sync.dma_start(
                        A[6][0:96, 32 * WP: 34 * WP], A[6][32:128, 0:2 * WP]))
                    hcnt[6] += 16
                    inc(i, 'sh6', 16)
                    halo6 = ('sh6', hcnt[6])

            # ======== L6: G4, M=8, two row-tiles ========
            l6_evacs = []
            for tau in range(2):
                slot = tile_g % 2; tile_g += 1
                PS = P[slot]
                pe_tile_begin(slot, waits_other=[l5_evacs[tau]])
                for j in range(4):
                    if j == 3:
                        if tau == 0:
                            wait('pe', l5_evacs[1][0], l5_evacs[1][1])
                        else:
                            wait('pe', halo6[0], halo6[1])
                    for p, (tA, tB) in enumerate(PAIRS):
                        kiA, kjA = tA
                        delta = 128 if tB else 16
                        dr_matmul(PS, j * 512, OFF[6] + p * 32, 8,
                                  A[6], (16 * tau + 4 * j + kiA) * WP + kjA,
                                  delta, 508,
                                  start=(p == 0), stop=(p == 5),
                                  final=(p == 5 and j == 3))
                wait('act', 'spe', cnt['pe'])
                if tau == 0 and out_cnt:
                    wait('act', 'sout', out_cnt)
                def mk6(PS=PS, tau=tau):
                    sv = PS[0:8, 0:3 * 512 + 508]
                    sv = dataclasses.replace(
                        sv, ap=[sv.ap[0], [512, 4], [127, 4], [1, 115]])
                    dv = OUTB[0:8, tau * 16 * 115: tau * 16 * 115 + 16 * 115]
                    return nc.scalar.activation(dv, sv, AF.Sign)
                i = emit('act', mk6)
                cnt['act'] += 1
                inc(i, 'sact', 1)
                efree = ('sact', cnt['act'])
                slot_free[slot] = efree
                l6_evacs.append(efree)

            # ======== output DMA: 8 flat per-(band,ch) planes (fp8) ========
            wait('sp', l6_evacs[1][0], l6_evacs[1][1])
            for g in range(4):
                nrows = 32 if g < 3 else 19
                for c in range(2):
                    def mko(img=img, g=g, c=c, nrows=nrows):
                        sv = OUTB[2 * g + c: 2 * g + c + 1, 0: nrows * 115]
                        dv = y[img, 0:1]
                        dv = dataclasses.replace(
                            dv, offset=dv.offset + c * 13225 + 32 * g * 115,
                            ap=[[1, nrows * 115]])
                        return nc.sync.dma_start(dv, sv)
                    i = emit('sp', mko)
                    out_cnt += 16
                    inc(i, 'sout', 16)
        return cnt

    with nc.Block() as block:
        @block.tensor
        def _(E):
            walk(E, 'pe')

        @block.scalar
        def _(E):
            walk(E, 'act')

        @block.vector
        def _(E):
            walk(E, 'dve')

        @block.gpsimd
        def _(E):
            walk(E, 'gp')

        @block.sync
        def _(E):
            walk(E, 'sp')

    for cm in reversed(ctxs):
        cm.__exit__(None, None, None)
    return nc


def round11(x):
    """Round fp32 array to 11-bit significand (RNE on low 13 mantissa bits)."""
    b = x.view(np.uint32).copy()
    low = b & np.uint32(0x1FFF)
    base = b & ~np.uint32(0x1FFF)
    rnd = (low > 0x1000) | ((low == 0x1000) & ((b >> 13) & 1).astype(bool))
    base = base + (rnd.astype(np.uint32) << 13)
    return base.view(np.float32)


def split_input(inp):
    """(N,3,256,256) fp32 -> (N,2,3,256,256): two 11-bit fp32r terms."""
    t1 = round11(inp)
    t2 = round11((inp - t1).astype(np.float32))
    return np.stack([t1, t2], axis=1)


def pack_weights(ws):
    """ws: 7 raw arrays (cout, cin, 3, 3) -> (w0f fp32, wf8 fp8)."""
    import ml_dtypes
    sws = [np.sign(w).astype(np.float32) for w in ws]
    # L0: 32 bands x 3cin on 96 partitions -> 128 out (32 bands x 4)
    w0f = np.zeros((96, 9 * 128), np.float32)
    for tap in range(9):
        ki, kj = tap // 3, tap % 3
        blk = sws[0][:, :, ki, kj].T  # (cin, cout)
        for s in range(32):
            w0f[s * 3:s * 3 + 3, tap * 128 + s * 4: tap * 128 + s * 4 + 4] = blk
    wf8 = np.zeros((128, WF8_COLS), np.float32)
    # t2 layers: 1,2,3 (phases), 4 (cout halves), 6 (plain)
    for l, nph in ((1, 2), (2, 2), (3, 2)):
        cin, cout = CH[l]
        gin = G[l]
        M = 128
        for p, (tA, tB) in enumerate(PAIRS):
            for ph in range(nph):
                col = OFF[l] + (p * 2 + ph) * 256
                for i, tap in enumerate((tA, tB)):
                    if tap is None:
                        continue
                    ki, kj = tap
                    blk = sws[l][:, :, ki, kj].T  # (cin, cout)
                    for gp_ in range(gin // 2):
                        g = 2 * gp_ + ph
                        wf8[g * cin:(g + 1) * cin,
                            col + i * M + gp_ * cout: col + i * M + (gp_ + 1) * cout] = blk
    # L4: cout halves
    cin, cout = CH[4]
    for p, (tA, tB) in enumerate(PAIRS):
        for h in range(2):
            col = OFF[4] + (p * 2 + h) * 256
            for i, tap in enumerate((tA, tB)):
                if tap is None:
                    continue
                ki, kj = tap
                blk = sws[4][32 * h:32 * h + 32, :, ki, kj].T  # (32cin, 32cout)
                for g in range(4):
                    wf8[g * 32:(g + 1) * 32,
                        col + i * 128 + g * 32: col + i * 128 + (g + 1) * 32] = blk
    # L5: t1 k-split (slab i = channels 32i..32i+32)
    for tap in range(9):
        ki, kj = tap // 3, tap % 3
        col = OFF[5] + tap * 256
        for i in range(2):
            blk = sws[5][:, 32 * i:32 * i + 32, ki, kj].T  # (32cin-half, 32cout)
            for g in range(4):
                wf8[g * 32:(g + 1) * 32,
                    col + i * 128 + g * 32: col + i * 128 + (g + 1) * 32] = blk
    # L6: M=8 (ktile step padded to 16)
    for p, (tA, tB) in enumerate(PAIRS):
        col = OFF[6] + p * 32
        for i, tap in enumerate((tA, tB)):
            if tap is None:
                continue
            ki, kj = tap
            blk = sws[6][:, :, ki, kj].T  # (32, 2)
            for g in range(4):
                wf8[g * 32:(g + 1) * 32,
                    col + i * 16 + g * 2: col + i * 16 + (g + 1) * 2] = blk
    return w0f, wf8.astype(ml_dtypes.float8_e4m3fn)


LAST_RESULTS = None


def kernel(**inputs):
    global LAST_RESULTS
    from concourse.bass_utils import run_bass_kernel_spmd
    inp = np.asarray(inputs['inputs'], np.float32)
    ws = [np.asarray(inputs[f'w{i}']) for i in range(7)]
    w0f, wf8 = pack_weights(ws)
    nc = build_program()
    in_maps = []
    for c in range(8):
        xs = split_input(np.ascontiguousarray(inp[c * 8:(c + 1) * 8]))
        in_maps.append({'x': np.ascontiguousarray(xs),
                        'w0f': w0f, 'wf8': wf8})
    res = run_bass_kernel_spmd(nc, in_maps, core_ids=list(range(8)),
                               tmpdir=os.environ.get('KERNEL_TRACE_DIR') or None)
    LAST_RESULTS = res
    out = np.concatenate([np.asarray(res.results[c]['y'], np.float32)
                          for c in range(8)], axis=0)
    return out


# revision 11
# speedup vs baseline: 1.1879x; 1.1342x over previous
"""Trainium2 Bass kernel v3 for the 7-layer binarized CNN (nn_MCNET).

Data parallel over 8 cores (8 images each). Per core:
- L0 (3->4, fp32 input): input split HOST-SIDE into two 11-bit-significand
  fp32 terms (t1 = round11(x), t2 = round11(x - t1), residual <= 2^-22|x|),
  fed to the PE as float32r (TRN2 PE keeps exactly 11 mantissa bits on the
  fp32r moving path, so both terms pass through exactly; verified on HW).
  18 accumulating fp32r matmuls per psum chunk (vs 27 bf16 in v2) over
  block-diagonal weights (32 row-bands x 3cin on 96 partitions -> 128 out).
  No on-device decomposition: Pool only does init memsets.
- Per-psum-chunk ACT Sign -> bf16, DVE 2x2 maxpool writes fp8 directly
  into A1's banded layout.
- L1..L6: fp8e4 DoubleRow matmuls. Activations live in per-layer banded
  buffers A_l: G bands x cin channels on 128 partitions, rows contiguous at
  stride 127 (127 % 16 == 15, so tap pairs (ki,kj)->(ki+1,kj+1) have ktile
  stride 128, a legal DoubleRow step). Tap-paired layers run 6 DoubleRow
  passes instead of 9; L5 (cin=64) k-splits channels across two 4320-byte
  slabs instead. Band nesting is chosen so every PSUM evacuation is an
  identity-partition ACT/DVE op (sign == clip for the even-integer sums),
  and inter-band halos are ONE contiguous partition-shifted SBUF DMA per
  layer. All DMAs issue from the SP (sync) engine -> HWDGE.
- Output stored/DMA'd as fp8e4 (values in {-1,0,1} exact), converted to
  fp32 on host.
"""
import sys, os, dataclasses
sys.path.insert(0, '/opt/trn_rl_repo')
import numpy as np

CH = [(3, 4), (4, 8), (8, 16), (16, 32), (32, 64), (64, 32), (32, 2)]
HIN = [256, 127, 125, 123, 121, 119, 117]
HOUT = [h - 2 for h in HIN]
G = [32, 32, 16, 8, 4, 4, 4]         # in-bands per layer
BB = [8, 4, 8, 16, 32, 32, 32]       # nominal in-band rows
WP = 127                              # fp8 row stride (127 % 16 == 15)
SL5 = 4320                            # A5 slab stride (34*127=4318 -> pad to %16)
NIMG = 8
# tap pairs for t2 (tap-paired DoubleRow): ktile delta = 127*dki + dkj = 128
PAIRS = [((0, 0), (1, 1)), ((0, 1), (1, 2)), ((1, 0), (2, 1)),
         ((0, 2), None), ((2, 0), None), ((2, 2), None)]
# weight block column offsets in WF8
NB = [0, 12, 12, 12, 12, 9, 6]        # lhsT blocks per layer (l1..l6 used)
OFF = {}
_c = 0
for _l in range(1, 7):
    OFF[_l] = _c
    _c += NB[_l] * (256 if _l < 6 else 32)
WF8_COLS = _c
A_ROWS = [0, 6, 10, 18, 34, 0, 34]    # stored rows per band (A5 special)
A_COLS = [0] + [A_ROWS[l] * WP + 384 for l in range(1, 7)]
A_COLS[5] = 2 * SL5 + 384
TSLOT = 5120                          # T cols per slot: 2 terms x 10 rows x 256


def build_program():
    import concourse.bass as bass
    import concourse.mybir as mybir
    dt = mybir.dt
    AF = mybir.ActivationFunctionType
    PM = mybir.MatmulPerfMode
    ALU = mybir.AluOpType

    nc = bass.Bass("TRN2", target_bir_lowering=False)
    x = nc.dram_tensor("x", (NIMG, 2, 3, 258, 256), dt.float32r,
                       kind="ExternalInput")
    w0f = nc.dram_tensor("w0f", (96, 9 * 128), dt.float32r, kind="ExternalInput")
    wf8 = nc.dram_tensor("wf8", (128, WF8_COLS), dt.float8e4, kind="ExternalInput")
    z8 = nc.dram_tensor("z8", (128, 9024), dt.float8e4, kind="ExternalInput")
    y = nc.dram_tensor("y", (NIMG, 2 * 115 * 115), dt.float8e4,
                       kind="ExternalOutput")

    ctxs = []
    def alloc(cm):
        ctxs.append(cm)
        return cm.__enter__()

    W0F = alloc(nc.sbuf_tensor("W0F", [96, 9 * 128], dt.float32r))
    WF8 = alloc(nc.sbuf_tensor("WF8", [128, WF8_COLS], dt.float8e4))
    T = alloc(nc.sbuf_tensor("T", [96, 2 * TSLOT], dt.float32r))
    A = [None] * 7
    for l in range(1, 7):
        A[l] = alloc(nc.sbuf_tensor(f"A{l}", [128, A_COLS[l]], dt.float8e4))
    T0B = alloc(nc.sbuf_tensor("T0B", [128, 2032], dt.bfloat16))
    T1B = alloc(nc.sbuf_tensor("T1B", [128, 4 * 254], dt.bfloat16))
    OUTB = alloc(nc.sbuf_tensor("OUTB", [128, 32 * 115], dt.float8e4))
    P = [alloc(nc.psum_tensor(f"P{i}", [128, 2048], dt.float32)) for i in range(2)]
    sem = {n: alloc(nc.semaphore(name=n)) for n in
           ['sdma', 'spe', 'sact', 'sdve', 'sgp', 'sin0', 'sin1', 'swf',
            'sh1', 'sh2', 'sh3', 'sh4', 'sh5', 'sh6', 'sout']}

    def walk(E, me):
        cnt = {'dma': 0, 'pe': 0, 'act': 0, 'dve': 0, 'gp': 0}
        last_wait = {}

        def wait(eng, semn, val):
            if val is None or val <= 0:
                return
            k = (eng, semn)
            if last_wait.get(k, -1) >= val:
                return
            last_wait[k] = val
            if eng == me:
                E.wait_ge(sem[semn], val)

        def emit(eng, fn):
            if eng == me:
                return fn()
            return None

        def inc(inst, semn, v):
            if inst is not None:
                inst.then_inc(sem[semn], v)

        # ---- init: zero A-layers via cheap DMAs from a dram zeros tensor ----
        # (input rows are host-padded to 258 with zeros, so T needs no init)
        # SP order: W0F, zA1, zA2, WF8, zA3..zA6 -- A1/A2 ready early, WF8 in
        # time for L1 of img0, big A5 fill done well before L4 of img0.
        i = emit('sp', lambda: nc.sync.dma_start(W0F[0:96, :], w0f[:]))
        inc(i, 'sdma', 16)
        ms_cnt = 0
        MS_A = {}
        def emit_zero(l):
            nonlocal ms_cnt
            i = emit('sp', lambda l=l: nc.sync.dma_start(
                A[l][0:128, 0:A_COLS[l]], z8[:, 0:A_COLS[l]]))
            ms_cnt += 16
            inc(i, 'sgp', 16)
            MS_A[l] = ms_cnt
        emit_zero(1)
        emit_zero(2)
        i = emit('sp', lambda: nc.sync.dma_start(WF8[0:128, :], wf8[:]))
        inc(i, 'swf', 16)
        for l in (3, 4, 5, 6):
            emit_zero(l)

        slot_free = [None, None]
        t_free = [None, None]        # spe count when T slot fully read
        vmax_prev = None            # sdve count of prev img vmax (T0B free)
        hcnt = {l: 0 for l in range(1, 7)}   # per-halo-sem cumulative counts
        out_cnt = 0
        tile_g = 0

        def pe_tile_begin(slot, waits_other=(), layer=1):
            if layer == 0:
                wait('pe', 'sdma', 16)      # W0F
            else:
                wait('pe', 'sgp', MS_A[layer])
                wait('pe', 'swf', 16)       # WF8
            for sname, v in waits_other:
                wait('pe', sname, v)
            if slot_free[slot] is not None:
                wait('pe', slot_free[slot][0], slot_free[slot][1])

        def dr_matmul(PS, psoff, lhs_col, lhs_m, rhs_buf, rhs_off, rhs_delta,
                      n, start, stop, final):
            lstep = max(16, lhs_m)
            def mk():
                lv = WF8[0:128, lhs_col: lhs_col + lstep + lhs_m]
                lv = dataclasses.replace(lv, ap=[lv.ap[0], [lstep, 2], [1, lhs_m]])
                rv = rhs_buf[0:128, rhs_off: rhs_off + rhs_delta + n]
                rv = dataclasses.replace(rv, ap=[rv.ap[0], [rhs_delta, 2], [1, n]])
                ov = PS[0:lhs_m, psoff: psoff + n]
                return nc.tensor.matmul(ov, lv, rv, start=start, stop=stop,
                                        perf_mode=PM.DoubleRow)
            i = emit('pe', mk)
            if final:
                cnt['pe'] += 1
                inc(i, 'spe', 1)
            return i

        def evac(eng, PS, nchunks, nlast, dstbuf, dstoff, dst_ms, mpart,
                 sdma_guard):
            wait(eng, 'spe', cnt['pe'])
            if dst_ms is not None:
                wait(eng, 'sgp', dst_ms)
            if sdma_guard is not None:
                wait(eng, sdma_guard[0], sdma_guard[1])
            total = 508 * (nchunks - 1) + nlast
            def mk():
                sv = PS[0:mpart, 0:(nchunks - 1) * 512 + nlast]
                sv = dataclasses.replace(sv, ap=[sv.ap[0], [512, nchunks], [1, 508]]) \
                    if nchunks > 1 else dataclasses.replace(sv, ap=[sv.ap[0], [1, nlast]])
                dv = dstbuf[0:mpart, dstoff: dstoff + total]
                if eng == 'act':
                    return nc.scalar.activation(dv, sv, AF.Sign)
                return nc.vector.tensor_scalar(dv, sv, 1.0, -1.0, ALU.min, ALU.max)
            i = emit(eng, mk)
            key = 'sact' if eng == 'act' else 'sdve'
            cnt[eng] += 1
            inc(i, key, 1)
            return (key, cnt[eng])

        dma_in_done = [None] * NIMG
        dma_in_t0 = [None] * NIMG
        in_cnt = [0, 0]

        def emit_in_dma(j):
            # input DMA for image j (slot j%2): one DMA per term on the Pool
            # (gpsimd) queue -- rows host-padded to 258 so all 32 bands load
            # in a single AP. Dedicated per-slot semaphore (race-free counts).
            aslot = j % 2
            sname = f'sin{aslot}'
            toff = aslot * TSLOT
            if t_free[aslot] is not None:
                wait('gp', 'spe', t_free[aslot])
            for t in range(2):
                src_main = dataclasses.replace(
                    x[j, t], ap=[[2048, 32], [66048, 3], [256, 10], [1, 256]])
                i = emit('gp', lambda src_main=src_main, toff=toff, t=t:
                         nc.gpsimd.dma_start(
                             T[0:96, toff + t * 2560: toff + t * 2560 + 2560],
                             src_main))
                in_cnt[aslot] += 1
                inc(i, sname, 16)
                if t == 0:
                    dma_in_t0[j] = (sname, in_cnt[aslot])
            dma_in_done[j] = (sname, in_cnt[aslot])

        for img in range(NIMG):
            # ======== input DMA prefetch (this img on img0, next img after) ====
            if img == 0:
                emit_in_dma(0)
            if img + 1 < NIMG:
                emit_in_dma(img + 1)
            dma_in = dma_in_done[img]
            aslot = img % 2
            toff = aslot * TSLOT

            # ======== L0: fp32r conv, 32 bands x 8 rows, 2 host-split terms ====
            slot = tile_g % 2; tile_g += 1
            PS = P[slot]
            pe_tile_begin(slot, layer=0)
            l0_chunk_pe = [None] * 4

            def l0_mm(c, t, tap, PS, final):
                ki, kj = tap // 3, tap % 3
                rbase = toff + t * 2560 + (2 * c + ki) * 256 + kj
                def mk(c=c, rbase=rbase, tap=tap, PS=PS, t=t):
                    lhsT = W0F[0:96, tap * 128: tap * 128 + 128]
                    rv = T[0:96, rbase: rbase + 256 + 254]
                    rv = dataclasses.replace(rv,
                                             ap=[rv.ap[0], [256, 2], [1, 254]])
                    ov = PS[0:128, c * 512: c * 512 + 508]
                    return nc.tensor.matmul(
                        ov, lhsT, rv, start=(t == 0 and tap == 0),
                        stop=(t == 1 and tap == 8))
                i = emit('pe', mk)
                if final:
                    cnt['pe'] += 1
                    inc(i, 'spe', 1)

            if img == 0:
                # term-outer: PE starts after term-0 DMA only
                for t in range(2):
                    dm = dma_in_t0[0] if t == 0 else dma_in
                    wait('pe', dm[0], 16 * dm[1])
                    for c in range(4):
                        for tap in range(9):
                            final = (t == 1 and tap == 8)
                            l0_mm(c, t, tap, PS, final)
                            if final:
                                l0_chunk_pe[c] = cnt['pe']
            else:
                wait('pe', dma_in[0], 16 * dma_in[1])
                for c in range(4):
                    for t in range(2):
                        for tap in range(9):
                            final = (t == 1 and tap == 8)
                            l0_mm(c, t, tap, PS, final)
                            if final:
                                l0_chunk_pe[c] = cnt['pe']
            t_free[aslot] = cnt['pe']

            # per-chunk pool pipeline: sign(c) -> vmax(c) -> hmax(c)
            vmax_cur = []
            for c in range(4):
                wait('act', 'spe', l0_chunk_pe[c])
                if vmax_prev is not None:
                    wait('act', 'sdve', vmax_prev[c])
                def mksgn(PS=PS, c=c):
                    return nc.scalar.activation(
                        T0B[0:128, c * 508: c * 508 + 508],
                        PS[0:128, c * 512: c * 512 + 508], AF.Sign)
                i = emit('act', mksgn)
                cnt['act'] += 1
                inc(i, 'sact', 1)
                if c == 3:
                    slot_free[slot] = ('sact', cnt['act'])
                wait('dve', 'sact', cnt['act'])
                def mkv(c=c):
                    a = T0B[0:128, c * 508: c * 508 + 254]
                    b = T0B[0:128, c * 508 + 254: c * 508 + 508]
                    return nc.vector.tensor_max(
                        T1B[0:128, c * 254: c * 254 + 254], a, b)
                i = emit('dve', mkv)
                cnt['dve'] += 1
                inc(i, 'sdve', 1)
                vmax_cur.append(cnt['dve'])
                wait('dve', 'sdve', cnt['dve'])   # RAW on T1B
                if c == 0:
                    wait('dve', 'sgp', MS_A[1])
                    wait('dve', 'sh1', hcnt[1])
                def mkh(c=c):
                    sv = T1B[0:128, c * 254: c * 254 + 254]
                    a = dataclasses.replace(sv, ap=[sv.ap[0], [2, 127]])
                    b = dataclasses.replace(sv, offset=sv.offset + 1,
                                            ap=[sv.ap[0], [2, 127]])
                    return nc.vector.tensor_max(
                        A[1][0:128, c * WP: c * WP + 127], a, b)
                i = emit('dve', mkh)
                cnt['dve'] += 1
                inc(i, 'sdve', 1)
                if c == 1:
                    hmax01 = cnt['dve']
            hmax_all = cnt['dve']
            vmax_prev = vmax_cur

            # halo A1 (SP): bands g+1 rows 0:2 -> band g rows 4:6
            wait('sp', 'sdve', hmax01)
            i = emit('sp', lambda: nc.sync.dma_start(
                A[1][0:124, 4 * WP: 6 * WP], A[1][4:128, 0:2 * WP]))
            hcnt[1] += 16
            inc(i, 'sh1', 16)
            halo1 = ('sh1', hcnt[1])

            # ======== L1: G32 -> 16 out-bands, 2 phase-chunks ========
            slot = tile_g % 2; tile_g += 1
            PS = P[slot]
            pe_tile_begin(slot, waits_other=[halo1, ('sdve', hmax_all)])
            for p, (tA, tB) in enumerate(PAIRS):
                for ph in range(2):
                    kiA, kjA = tA
                    delta = 128 if tB else 16
                    dr_matmul(PS, ph * 512, OFF[1] + (p * 2 + ph) * 256, 128,
                              A[1], kiA * WP + kjA, delta, 508,
                              start=(p == 0), stop=(p == 5),
                              final=(p == 5 and ph == 1))
            efree = evac('dve', PS, 2, 508, A[2], 0, MS_A[2], 128,
                         ('sh2', hcnt[2]) if hcnt[2] else None)
            l1_evac = efree
            slot_free[slot] = efree
            wait('sp', efree[0], efree[1])
            i = emit('sp', lambda: nc.sync.dma_start(
                A[2][0:120, 8 * WP: 10 * WP], A[2][8:128, 0:2 * WP]))
            hcnt[2] += 16
            inc(i, 'sh2', 16)
            halo2 = ('sh2', hcnt[2])

            # ======== L2: G16 -> 8, chunks (ph, j) ========
            slot = tile_g % 2; tile_g += 1
            PS = P[slot]
            pe_tile_begin(slot, waits_other=[l1_evac])
            for j in range(2):
                if j == 1:
                    wait('pe', halo2[0], halo2[1])
                for ph in range(2):
                    for p, (tA, tB) in enumerate(PAIRS):
                        kiA, kjA = tA
                        delta = 128 if tB else 16
                        dr_matmul(PS, (2 * ph + j) * 512,
                                  OFF[2] + (p * 2 + ph) * 256, 128,
                                  A[2], (4 * j + kiA) * WP + kjA, delta, 508,
                                  start=(p == 0), stop=(p == 5),
                                  final=(p == 5 and ph == 1 and j == 1))
            eh1 = evac('act', PS, 2, 508, A[3], 0, MS_A[3], 128,
                       ('sh3', hcnt[3]) if hcnt[3] else None)
            l2_evac_h1 = eh1
            def mk_l2e2(PS=PS):
                sv = PS[0:128, 2 * 512: 3 * 512 + 508]
                sv = dataclasses.replace(sv, ap=[sv.ap[0], [512, 2], [1, 508]])
                dv = A[3][0:128, 8 * WP: 16 * WP]
                return nc.scalar.activation(dv, sv, AF.Sign)
            i = emit('act', mk_l2e2)
            cnt['act'] += 1
            inc(i, 'sact', 1)
            efree = ('sact', cnt['act'])
            l2_evac = efree
            slot_free[slot] = efree
            # halo3 source rows 0,1 are in the h1 evac -> issue early
            wait('sp', l2_evac_h1[0], l2_evac_h1[1])
            i = emit('sp', lambda: nc.sync.dma_start(
                A[3][0:112, 16 * WP: 18 * WP], A[3][16:128, 0:2 * WP]))
            hcnt[3] += 16
            inc(i, 'sh3', 16)
            halo3 = ('sh3', hcnt[3])

            # ======== L3: G8 -> 4, two row-tiles (tau = phase) ========
            l3_evacs = []
            for tau in range(2):
                slot = tile_g % 2; tile_g += 1
                PS = P[slot]
                pe_tile_begin(slot, waits_other=[l2_evac_h1])
                for j in range(4):
                    if j == 1:
                        wait('pe', l2_evac[0], l2_evac[1])
                    if j == 3:
                        wait('pe', halo3[0], halo3[1])
                    for p, (tA, tB) in enumerate(PAIRS):
                        kiA, kjA = tA
                        delta = 128 if tB else 16
                        dr_matmul(PS, j * 512, OFF[3] + (p * 2 + tau) * 256, 128,
                                  A[3], (4 * j + kiA) * WP + kjA, delta, 508,
                                  start=(p == 0), stop=(p == 5),
                                  final=(p == 5 and j == 3))
                efree = evac('dve', PS, 4, 508, A[4], tau * 16 * WP, MS_A[4], 128,
                             ('sh4', hcnt[4]) if (tau == 0 and hcnt[4]) else None)
                slot_free[slot] = efree
                l3_evacs.append(efree)
                if tau == 0:
                    wait('sp', efree[0], efree[1])
                    i = emit('sp', lambda: nc.sync.dma_start(
                        A[4][0:96, 32 * WP: 34 * WP], A[4][32:128, 0:2 * WP]))
                    hcnt[4] += 16
                    inc(i, 'sh4', 16)
                    halo4 = ('sh4', hcnt[4])

            # ======== L4: G4, tiles (h, tau) ========
            l4_t0_evacs = []
            l4_evac_last = None
            for h in range(2):
                for tau in range(2):
                    slot = tile_g % 2; tile_g += 1
                    PS = P[slot]
                    pe_tile_begin(slot, waits_other=[l3_evacs[tau]])
                    for j in range(4):
                        if j == 3:
                            if tau == 0:
                                wait('pe', l3_evacs[1][0], l3_evacs[1][1])
                            else:
                                wait('pe', halo4[0], halo4[1])
                        for p, (tA, tB) in enumerate(PAIRS):
                            kiA, kjA = tA
                            delta = 128 if tB else 16
                            dr_matmul(PS, j * 512, OFF[4] + (p * 2 + h) * 256, 128,
                                      A[4], (16 * tau + 4 * j + kiA) * WP + kjA,
                                      delta, 508,
                                      start=(p == 0), stop=(p == 5),
                                      final=(p == 5 and j == 3))
                    efree = evac('act', PS, 4, 508, A[5],
                                 h * SL5 + tau * 16 * WP, MS_A[5], 128,
                                 ('sh5', hcnt[5]) if (h == 0 and tau == 0 and hcnt[5]) else None)
                    slot_free[slot] = efree
                    l4_evac_last = efree
                    if tau == 0:
                        l4_t0_evacs.append(efree)
            wait('sp', l4_t0_evacs[1][0], l4_t0_evacs[1][1])
            def mkh5():
                sv = A[5][32:128, 0:SL5 + 2 * WP]
                sv = dataclasses.replace(sv, ap=[sv.ap[0], [SL5, 2], [1, 2 * WP]])
                dv = A[5][0:96, 32 * WP: SL5 + 34 * WP]
                dv = dataclasses.replace(dv, ap=[dv.ap[0], [SL5, 2], [1, 2 * WP]])
                return nc.sync.dma_start(dv, sv)
            i = emit('sp', mkh5)
            hcnt[5] += 16
            inc(i, 'sh5', 16)
            halo5 = ('sh5', hcnt[5])

            # ======== L5: G4, t1 (k-split slabs), 9 taps, two row-tiles ========
            l5_evacs = []
            for tau in range(2):
                slot = tile_g % 2; tile_g += 1
                PS = P[slot]
                if tau == 0:
                    pe_tile_begin(slot, waits_other=[l4_t0_evacs[1]])
                else:
                    pe_tile_begin(slot, waits_other=[l4_evac_last])
                for j in range(4):
                    if j == 3:
                        if tau == 0:
                            wait('pe', l4_evac_last[0], l4_evac_last[1])
                        else:
                            wait('pe', halo5[0], halo5[1])
                    for tap in range(9):
                        ki, kj = tap // 3, tap % 3
                        dr_matmul(PS, j * 512, OFF[5] + tap * 256, 128,
                                  A[5], (16 * tau + 4 * j + ki) * WP + kj,
                                  SL5, 508,
                                  start=(tap == 0), stop=(tap == 8),
                                  final=(tap == 8 and j == 3))
                efree = evac('dve', PS, 4, 508, A[6], tau * 16 * WP, MS_A[6], 128,
                             ('sh6', hcnt[6]) if (tau == 0 and hcnt[6]) else None)
                slot_free[slot] = efree
                l5_evacs.append(efree)
                if tau == 0:
                    wait('sp', efree[0], efree[1])
                    i = emit('sp', lambda: nc.sync.dma_start(
                        A[6][0:96, 32 * WP: 34 * WP], A[6][32:128, 0:2 * WP]))
                    hcnt[6] += 16
                    inc(i, 'sh6', 16)
                    halo6 = ('sh6', hcnt[6])

            # ======== L6: G4, M=8, two row-tiles ========
            l6_evacs = []
            for tau in range(2):
                slot = tile_g % 2; tile_g += 1
                PS = P[slot]
                pe_tile_begin(slot, waits_other=[l5_evacs[tau]])
                for j in range(4):
                    if j == 3:
                        if tau == 0:
                            wait('pe', l5_evacs[1][0], l5_evacs[1][1])
                        else:
                            wait('pe', halo6[0], halo6[1])
                    for p, (tA, tB) in enumerate(PAIRS):
                        kiA, kjA = tA
                        delta = 128 if tB else 16
                        dr_matmul(PS, j * 512, OFF[6] + p * 32, 8,
                                  A[6], (16 * tau + 4 * j + kiA) * WP + kjA,
                                  delta, 508,
                                  start=(p == 0), stop=(p == 5),
                                  final=(p == 5 and j == 3))
                wait('act', 'spe', cnt['pe'])
                if tau == 0 and out_cnt:
                    wait('act', 'sout', out_cnt)
                def mk6(PS=PS, tau=tau):
                    sv = PS[0:8, 0:3 * 512 + 508]
                    sv = dataclasses.replace(
                        sv, ap=[sv.ap[0], [512, 4], [127, 4], [1, 115]])
                    dv = OUTB[0:8, tau * 16 * 115: tau * 16 * 115 + 16 * 115]
                    return nc.scalar.activation(dv, sv, AF.Sign)
                i = emit('act', mk6)
                cnt['act'] += 1
                inc(i, 'sact', 1)
                efree = ('sact', cnt['act'])
                slot_free[slot] = efree
                l6_evacs.append(efree)

            # ======== output DMA: 8 flat per-(band,ch) planes (fp8) ========
            wait('sp', l6_evacs[1][0], l6_evacs[1][1])
            for g in range(4):
                nrows = 32 if g < 3 else 19
                for c in range(2):
                    def mko(img=img, g=g, c=c, nrows=nrows):
                        sv = OUTB[2 * g + c: 2 * g + c + 1, 0: nrows * 115]
                        dv = y[img, 0:1]
                        dv = dataclasses.replace(
                            dv, offset=dv.offset + c * 13225 + 32 * g * 115,
                            ap=[[1, nrows * 115]])
                        return nc.sync.dma_start(dv, sv)
                    i = emit('sp', mko)
                    out_cnt += 16
                    inc(i, 'sout', 16)
        return cnt

    with nc.Block() as block:
        @block.tensor
        def _(E):
            walk(E, 'pe')

        @block.scalar
        def _(E):
            walk(E, 'act')

        @block.vector
        def _(E):
            walk(E, 'dve')

        @block.gpsimd
        def _(E):
            walk(E, 'gp')

        @block.sync
        def _(E):
            walk(E, 'sp')

    for cm in reversed(ctxs):
        cm.__exit__(None, None, None)
    return nc


def round11(x):
    """Round fp32 array to 11-bit significand (RNE on low 13 mantissa bits)."""
    b = x.view(np.uint32).copy()
    low = b & np.uint32(0x1FFF)
    base = b & ~np.uint32(0x1FFF)
    rnd = (low > 0x1000) | ((low == 0x1000) & ((b >> 13) & 1).astype(bool))
    base = base + (rnd.astype(np.uint32) << 13)
    return base.view(np.float32)


def split_input(inp):
    """(N,3,256,256) fp32 -> (N,2,3,258,256): two 11-bit fp32r terms,
    rows zero-padded to 258 so band 31's halo loads in the main DMA."""
    t1 = round11(inp)
    t2 = round11((inp - t1).astype(np.float32))
    xs = np.zeros((inp.shape[0], 2, 3, 258, 256), np.float32)
    xs[:, 0, :, :256] = t1
    xs[:, 1, :, :256] = t2
    return xs


def pack_weights(ws):
    """ws: 7 raw arrays (cout, cin, 3, 3) -> (w0f fp32, wf8 fp8)."""
    import ml_dtypes
    sws = [np.sign(w).astype(np.float32) for w in ws]
    # L0: 32 bands x 3cin on 96 partitions -> 128 out (32 bands x 4)
    w0f = np.zeros((96, 9 * 128), np.float32)
    for tap in range(9):
        ki, kj = tap // 3, tap % 3
        blk = sws[0][:, :, ki, kj].T  # (cin, cout)
        for s in range(32):
            w0f[s * 3:s * 3 + 3, tap * 128 + s * 4: tap * 128 + s * 4 + 4] = blk
    wf8 = np.zeros((128, WF8_COLS), np.float32)
    # t2 layers: 1,2,3 (phases), 4 (cout halves), 6 (plain)
    for l, nph in ((1, 2), (2, 2), (3, 2)):
        cin, cout = CH[l]
        gin = G[l]
        M = 128
        for p, (tA, tB) in enumerate(PAIRS):
            for ph in range(nph):
                col = OFF[l] + (p * 2 + ph) * 256
                for i, tap in enumerate((tA, tB)):
                    if tap is None:
                        continue
                    ki, kj = tap
                    blk = sws[l][:, :, ki, kj].T  # (cin, cout)
                    for gp_ in range(gin // 2):
                        g = 2 * gp_ + ph
                        wf8[g * cin:(g + 1) * cin,
                            col + i * M + gp_ * cout: col + i * M + (gp_ + 1) * cout] = blk
    # L4: cout halves
    cin, cout = CH[4]
    for p, (tA, tB) in enumerate(PAIRS):
        for h in range(2):
            col = OFF[4] + (p * 2 + h) * 256
            for i, tap in enumerate((tA, tB)):
                if tap is None:
                    continue
                ki, kj = tap
                blk = sws[4][32 * h:32 * h + 32, :, ki, kj].T  # (32cin, 32cout)
                for g in range(4):
                    wf8[g * 32:(g + 1) * 32,
                        col + i * 128 + g * 32: col + i * 128 + (g + 1) * 32] = blk
    # L5: t1 k-split (slab i = channels 32i..32i+32)
    for tap in range(9):
        ki, kj = tap // 3, tap % 3
        col = OFF[5] + tap * 256
        for i in range(2):
            blk = sws[5][:, 32 * i:32 * i + 32, ki, kj].T  # (32cin-half, 32cout)
            for g in range(4):
                wf8[g * 32:(g + 1) * 32,
                    col + i * 128 + g * 32: col + i * 128 + (g + 1) * 32] = blk
    # L6: M=8 (ktile step padded to 16)
    for p, (tA, tB) in enumerate(PAIRS):
        col = OFF[6] + p * 32
        for i, tap in enumerate((tA, tB)):
            if tap is None:
                continue
            ki, kj = tap
            blk = sws[6][:, :, ki, kj].T  # (32, 2)
            for g in range(4):
                wf8[g * 32:(g + 1) * 32,
                    col + i * 16 + g * 2: col + i * 16 + (g + 1) * 2] = blk
    return w0f, wf8.astype(ml_dtypes.float8_e4m3fn)


LAST_RESULTS = None


def kernel(**inputs):
    global LAST_RESULTS
    from concourse.bass_utils import run_bass_kernel_spmd
    inp = np.asarray(inputs['inputs'], np.float32)
    ws = [np.asarray(inputs[f'w{i}']) for i in range(7)]
    w0f, wf8 = pack_weights(ws)
    nc = build_program()
    import ml_dtypes
    z8 = np.zeros((128, 9024), ml_dtypes.float8_e4m3fn)
    in_maps = []
    for c in range(8):
        xs = split_input(np.ascontiguousarray(inp[c * 8:(c + 1) * 8]))
        in_maps.append({'x': np.ascontiguousarray(xs),
                        'w0f': w0f, 'wf8': wf8, 'z8': z8})
    res = run_bass_kernel_spmd(nc, in_maps, core_ids=list(range(8)),
                               tmpdir=os.environ.get('KERNEL_TRACE_DIR') or None)
    LAST_RESULTS = res
    out = np.concatenate([np.asarray(res.results[c]['y'], np.float32)
                          for c in range(8)], axis=0)
    return out


# revision 14
# speedup vs baseline: 1.2317x; 1.0368x over previous
"""Trainium2 Bass kernel v4 for the 7-layer binarized CNN (nn_MCNET).

Data parallel over 8 cores (8 images each). Per core:
- L0 (3->4, fp32 input): input split HOST-SIDE into two 11-bit-significand
  fp32 terms (t1 = round11(x), t2 = round11(x - t1), residual <= 2^-22|x|),
  fed to the PE as float32r (TRN2 PE keeps exactly 11 mantissa bits on the
  fp32r moving path, so both terms pass through exactly; verified on HW).
  18 accumulating fp32r matmuls per psum chunk over block-diagonal weights
  (32 row-bands x 3cin on 96 partitions -> 128 out). No on-device
  decomposition.
- Cross-image software pipelining: image j+1's four L0 chunks are emitted
  into image j's early-layer stall points (L1/L2/L3 evac+halo latencies),
  so the PE never drains between layers. PSUM is managed as 4 bank-pairs
  with a round-robin allocator; every multi-chunk tile evacuates per pair
  (two ACT/DVE ops) so pairs free mid-tile.
- Per-psum-chunk ACT Sign -> bf16, DVE 2x2 maxpool writes fp8 directly
  into A1's banded layout.
- L1..L6: fp8e4 DoubleRow matmuls. Activations live in per-layer banded
  buffers A_l: G bands x cin channels on 128 partitions, rows contiguous at
  stride 127 (tap pairs (ki,kj)->(ki+1,kj+1) have ktile stride 128, a legal
  DoubleRow step) -> 6 DoubleRow passes instead of 9; L5 (cin=64) k-splits
  channels across two 4320-byte slabs. Inter-band halos are single
  partition-shifted SBUF DMAs (SP queue); input DMAs ride the idle Pool
  queue; A-layer zero-fill comes from a dram zeros tensor (cheap DMAs).
- Output stored/DMA'd as fp8e4 per row-tile (values in {-1,0,1} exact),
  converted to fp32 on host.
"""
import sys, os, dataclasses
sys.path.insert(0, '/opt/trn_rl_repo')
import numpy as np

CH = [(3, 4), (4, 8), (8, 16), (16, 32), (32, 64), (64, 32), (32, 2)]
WP = 127                              # fp8 row stride (127 % 16 == 15)
SL5 = 4320                            # A5 slab stride (34*127=4318 -> pad to %16)
NIMG = 8
PAIRS = [((0, 0), (1, 1)), ((0, 1), (1, 2)), ((1, 0), (2, 1)),
         ((0, 2), None), ((2, 0), None), ((2, 2), None)]
NB = [0, 12, 12, 12, 12, 9, 6]        # lhsT blocks per layer (l1..l6 used)
OFF = {}
_c = 0
for _l in range(1, 7):
    OFF[_l] = _c
    _c += NB[_l] * (256 if _l < 6 else 32)
WF8_COLS = _c
A_ROWS = [0, 6, 10, 18, 34, 0, 34]
A_COLS = [0] + [A_ROWS[l] * WP + 384 for l in range(1, 7)]
A_COLS[5] = 2 * SL5 + 384
TSLOT = 5120                          # T cols per slot: 2 terms x 10 rows x 256


def build_program():
    import concourse.bass as bass
    import concourse.mybir as mybir
    dt = mybir.dt
    AF = mybir.ActivationFunctionType
    PM = mybir.MatmulPerfMode
    ALU = mybir.AluOpType

    nc = bass.Bass("TRN2", target_bir_lowering=False)
    x = nc.dram_tensor("x", (NIMG, 2, 3, 258, 256), dt.float32r,
                       kind="ExternalInput")
    w0f = nc.dram_tensor("w0f", (96, 9 * 128), dt.float32r, kind="ExternalInput")
    wf8 = nc.dram_tensor("wf8", (128, WF8_COLS), dt.float8e4, kind="ExternalInput")
    z8 = nc.dram_tensor("z8", (128, 9024), dt.float8e4, kind="ExternalInput")
    y = nc.dram_tensor("y", (NIMG, 2 * 115 * 115), dt.float8e4,
                       kind="ExternalOutput")

    ctxs = []
    def alloc(cm):
        ctxs.append(cm)
        return cm.__enter__()

    W0F = alloc(nc.sbuf_tensor("W0F", [96, 9 * 128], dt.float32r))
    WF8 = alloc(nc.sbuf_tensor("WF8", [128, WF8_COLS], dt.float8e4))
    T = alloc(nc.sbuf_tensor("T", [96, 2 * TSLOT], dt.float32r))
    A = [None] * 7
    for l in range(1, 7):
        A[l] = alloc(nc.sbuf_tensor(f"A{l}", [128, A_COLS[l]], dt.float8e4))
    T0B = alloc(nc.sbuf_tensor("T0B", [128, 2032], dt.bfloat16))
    T1B = alloc(nc.sbuf_tensor("T1B", [128, 4 * 254], dt.bfloat16))
    OUTB = alloc(nc.sbuf_tensor("OUTB", [128, 32 * 115], dt.float8e4))
    PS = alloc(nc.psum_tensor("PS", [128, 4096], dt.float32))
    sem = {n: alloc(nc.semaphore(name=n)) for n in
           ['sdma', 'spe', 'sact', 'sdve', 'sgp', 'sin0', 'sin1', 'swf',
            'sh1', 'sh2', 'sh3', 'sh4', 'sh5', 'sh6', 'sout']}

    def walk(E, me):
        cnt = {'pe': 0, 'act': 0, 'dve': 0}
        last_wait = {}

        def wait(eng, semn, val):
            if val is None or val <= 0:
                return
            k = (eng, semn)
            if last_wait.get(k, -1) >= val:
                return
            last_wait[k] = val
            if eng == me:
                E.wait_ge(sem[semn], val)

        def emit(eng, fn):
            if eng == me:
                return fn()
            return None

        def inc(inst, semn, v):
            if inst is not None:
                inst.then_inc(sem[semn], v)

        # ---- init: weights + A-layer zero-fill on SP queue ----
        i = emit('sp', lambda: nc.sync.dma_start(W0F[0:96, :], w0f[:]))
        inc(i, 'sdma', 16)
        ms_cnt = 0
        MS_A = {}
        def emit_zero(l):
            nonlocal ms_cnt
            i = emit('sp', lambda l=l: nc.sync.dma_start(
                A[l][0:128, 0:A_COLS[l]], z8[:, 0:A_COLS[l]]))
            ms_cnt += 16
            inc(i, 'sgp', 16)
            MS_A[l] = ms_cnt
        emit_zero(1)
        emit_zero(2)
        i = emit('sp', lambda: nc.sync.dma_start(WF8[0:128, :], wf8[:]))
        inc(i, 'swf', 16)
        for l in (3, 4, 5, 6):
            emit_zero(l)

        # ---- psum pair allocator: 4 pairs of 2 banks (1024 cols each) ----
        pair_free = [None] * 4
        pair_ptr = [0]

        def take_pair():
            p = pair_ptr[0]
            pair_ptr[0] = (p + 1) % 4
            if pair_free[p] is not None:
                wait('pe', pair_free[p][0], pair_free[p][1])
            return p

        hcnt = {l: 0 for l in range(1, 7)}
        out_cnt = [0]
        in_cnt = [0, 0]
        t_free = {}           # img -> spe count when its T slot fully read
        S = [dict() for _ in range(NIMG)]   # per-image state

        def emit_in_dma(j):
            # one DMA per term on the Pool queue (img0: term0 in 2 halves)
            aslot = j % 2
            sname = f'sin{aslot}'
            toff = aslot * TSLOT
            if j >= 2:
                wait('gp', 'spe', t_free[j - 2])
            pieces = []
            if j == 0:
                pieces = [(0, 0, 5), (0, 5, 5), (1, 0, 10)]
            else:
                pieces = [(0, 0, 10), (1, 0, 10)]
            marks = []
            for t, r0, nr in pieces:
                src = dataclasses.replace(
                    x[j, t],
                    offset=x[j, t].offset + r0 * 256,
                    ap=[[2048, 32], [66048, 3], [256, nr], [1, 256]])
                dst0 = toff + t * 2560 + r0 * 256
                i = emit('gp', lambda src=src, dst0=dst0, nr=nr:
                         nc.gpsimd.dma_start(T[0:96, dst0: dst0 + nr * 256], src))
                in_cnt[aslot] += 1
                inc(i, sname, 16)
                marks.append((sname, in_cnt[aslot]))
            S[j]['dma_marks'] = marks

        def l0_mm(jj, c, t, tap, pbase, final):
            aslot = jj % 2
            toff = aslot * TSLOT
            ki, kj = tap // 3, tap % 3
            rbase = toff + t * 2560 + (2 * c + ki) * 256 + kj
            def mk(rbase=rbase, tap=tap, pbase=pbase, t=t):
                lhsT = W0F[0:96, tap * 128: tap * 128 + 128]
                rv = T[0:96, rbase: rbase + 256 + 254]
                rv = dataclasses.replace(rv, ap=[rv.ap[0], [256, 2], [1, 254]])
                ov = PS[0:128, pbase: pbase + 508]
                return nc.tensor.matmul(
                    ov, lhsT, rv, start=(t == 0 and tap == 0),
                    stop=(t == 1 and tap == 8))
            i = emit('pe', mk)
            if final:
                cnt['pe'] += 1
                inc(i, 'spe', 1)

        def l0_chunk_evac(jj, c, pbase, pair):
            # sign (ACT) -> vmax -> hmax (DVE); halo1 after c==1
            st = S[jj]
            wait('act', 'spe', st['l0_pe'][c])
            if jj > 0 and 'vmax' in S[jj - 1]:
                wait('act', 'sdve', S[jj - 1]['vmax'][c])
            def mksgn(pbase=pbase, c=c):
                return nc.scalar.activation(
                    T0B[0:128, c * 508: c * 508 + 508],
                    PS[0:128, pbase: pbase + 508], AF.Sign)
            i = emit('act', mksgn)
            cnt['act'] += 1
            inc(i, 'sact', 1)
            pair_free[pair] = ('sact', cnt['act'])
            wait('dve', 'sact', cnt['act'])
            def mkv(c=c):
                a = T0B[0:128, c * 508: c * 508 + 254]
                b = T0B[0:128, c * 508 + 254: c * 508 + 508]
                return nc.vector.tensor_max(
                    T1B[0:128, c * 254: c * 254 + 254], a, b)
            i = emit('dve', mkv)
            cnt['dve'] += 1
            inc(i, 'sdve', 1)
            st.setdefault('vmax', {})[c] = cnt['dve']
            wait('dve', 'sdve', cnt['dve'])   # RAW on T1B
            wait('dve', 'sgp', MS_A[1])
            wait('dve', 'sh1', hcnt[1])
            if jj > 0:
                wait('dve', 'spe', S[jj - 1].get('l1_done'))
            def mkh(c=c):
                sv = T1B[0:128, c * 254: c * 254 + 254]
                a = dataclasses.replace(sv, ap=[sv.ap[0], [2, 127]])
                b = dataclasses.replace(sv, offset=sv.offset + 1,
                                        ap=[sv.ap[0], [2, 127]])
                return nc.vector.tensor_max(
                    A[1][0:128, c * WP: c * WP + 127], a, b)
            i = emit('dve', mkh)
            cnt['dve'] += 1
            inc(i, 'sdve', 1)
            if c == 1:
                # halo A1: bands g+1 rows 0:2 -> band g rows 4:6
                wait('sp', 'sdve', cnt['dve'])
                i = emit('sp', lambda: nc.sync.dma_start(
                    A[1][0:124, 4 * WP: 6 * WP], A[1][4:128, 0:2 * WP]))
                hcnt[1] += 16
                inc(i, 'sh1', 16)
                st['halo1'] = ('sh1', hcnt[1])
            if c == 3:
                st['hmax_all'] = cnt['dve']

        def emit_l0_chunk_steady(jj, c):
            # one L0 psum chunk (18 fp32r matmuls) + its evac chain
            st = S[jj]
            pair = take_pair()
            pbase = pair * 1024
            wait('pe', 'sdma', 16)      # W0F
            marks = st['dma_marks']
            wait('pe', marks[-1][0], 16 * marks[-1][1])
            for t in range(2):
                for tap in range(9):
                    final = (t == 1 and tap == 8)
                    l0_mm(jj, c, t, tap, pbase, final)
                    if final:
                        st['l0_pe'][c] = cnt['pe']
            if c == 3:
                t_free[jj] = cnt['pe']
            l0_chunk_evac(jj, c, pbase, pair)

        def emit_l0_img0():
            # prologue: all 4 chunks of img 0, term-outer, 2 pairs
            st = S[0]
            st['l0_pe'] = [None] * 4
            p0, p1 = take_pair(), take_pair()
            st['c_pbase'] = [p0 * 1024, p0 * 1024 + 512,
                             p1 * 1024, p1 * 1024 + 512]
            st['c_pair'] = [p0, p0, p1, p1]
            wait('pe', 'sdma', 16)
            marks = st['dma_marks']
            # emission: chunk loop lives in emit_l0_chunk(0, 0); evacs per chunk
            for t in range(2):
                if t == 1:
                    wait('pe', marks[2][0], 16 * marks[2][1])
                else:
                    wait('pe', marks[0][0], 16 * marks[0][1])
                for cc in range(4):
                    if t == 0 and cc == 2:
                        wait('pe', marks[1][0], 16 * marks[1][1])
                    for tap in range(9):
                        final = (t == 1 and tap == 8)
                        l0_mm(0, cc, t, tap, st['c_pbase'][cc], final)
                        if final:
                            st['l0_pe'][cc] = cnt['pe']
            t_free[0] = cnt['pe']
            for cc in range(4):
                l0_chunk_evac(0, cc, st['c_pbase'][cc], st['c_pair'][cc])

        def dr_matmul(pbase, lhs_col, lhs_m, rhs_buf, rhs_off, rhs_delta,
                      n, start, stop, final):
            lstep = max(16, lhs_m)
            def mk():
                lv = WF8[0:128, lhs_col: lhs_col + lstep + lhs_m]
                lv = dataclasses.replace(lv, ap=[lv.ap[0], [lstep, 2], [1, lhs_m]])
                rv = rhs_buf[0:128, rhs_off: rhs_off + rhs_delta + n]
                rv = dataclasses.replace(rv, ap=[rv.ap[0], [rhs_delta, 2], [1, n]])
                ov = PS[0:lhs_m, pbase: pbase + n]
                return nc.tensor.matmul(ov, lv, rv, start=start, stop=stop,
                                        perf_mode=PM.DoubleRow)
            i = emit('pe', mk)
            if final:
                cnt['pe'] += 1
                inc(i, 'spe', 1)
            return i

        def pair_evac(eng, pair, nchunks, dstbuf, dstoff, pe_count, dst_ms,
                      sdma_guard):
            # evacuate `nchunks` (1 or 2) 508-chunks of one pair -> contiguous
            wait(eng, 'spe', pe_count)
            if dst_ms is not None:
                wait(eng, 'sgp', dst_ms)
            if sdma_guard is not None:
                wait(eng, sdma_guard[0], sdma_guard[1])
            total = 508 * nchunks
            pbase = pair * 1024
            def mk():
                sv = PS[0:128, pbase: pbase + (nchunks - 1) * 512 + 508]
                sv = dataclasses.replace(
                    sv, ap=[sv.ap[0], [512, nchunks], [1, 508]]) \
                    if nchunks > 1 else dataclasses.replace(sv, ap=[sv.ap[0], [1, 508]])
                dv = dstbuf[0:128, dstoff: dstoff + total]
                if eng == 'act':
                    return nc.scalar.activation(dv, sv, AF.Sign)
                return nc.vector.tensor_scalar(dv, sv, 1.0, -1.0, ALU.min, ALU.max)
            i = emit(eng, mk)
            key = 'sact' if eng == 'act' else 'sdve'
            cnt[eng] += 1
            inc(i, key, 1)
            pair_free[pair] = (key, cnt[eng])
            return (key, cnt[eng])

        def emit_halo(l, dep, mk):
            wait('sp', dep[0], dep[1])
            i = emit('sp', mk)
            hcnt[l] += 16
            inc(i, f'sh{l}', 16)
            return (f'sh{l}', hcnt[l])

        # ---------------- per-tile emitters (layers 1..6) ----------------
        def emit_l1(j):
            st = S[j]
            pair = take_pair()
            pbase = pair * 1024
            wait('pe', 'sgp', MS_A[1])
            wait('pe', 'swf', 16)
            wait('pe', st['halo1'][0], st['halo1'][1])
            wait('pe', 'sdve', st['hmax_all'])
            for p, (tA, tB) in enumerate(PAIRS):
                for ph in range(2):
                    kiA, kjA = tA
                    delta = 128 if tB else 16
                    dr_matmul(pbase + ph * 512, OFF[1] + (p * 2 + ph) * 256, 128,
                              A[1], kiA * WP + kjA, delta, 508,
                              start=(p == 0), stop=(p == 5),
                              final=(p == 5 and ph == 1))
            st['l1_done'] = cnt['pe']
            ev = pair_evac('dve', pair, 2, A[2], 0, cnt['pe'], MS_A[2],
                           ('sh2', hcnt[2]) if hcnt[2] else None)
            st['l1_evac'] = ev
            st['halo2'] = emit_halo(2, ev, lambda: nc.sync.dma_start(
                A[2][0:120, 8 * WP: 10 * WP], A[2][8:128, 0:2 * WP]))

        def emit_l2(j):
            st = S[j]
            prs = [take_pair(), take_pair()]
            wait('pe', 'sgp', MS_A[2])
            wait('pe', st['l1_evac'][0], st['l1_evac'][1])
            pe_marks = []
            for k in range(4):          # chunk k = (ph, jj): ph=k//2, jj=k%2
                ph, jj = k // 2, k % 2
                if jj == 1:
                    wait('pe', st['halo2'][0], st['halo2'][1])
                pb = prs[k // 2] * 1024 + (k % 2) * 512
                for p, (tA, tB) in enumerate(PAIRS):
                    kiA, kjA = tA
                    delta = 128 if tB else 16
                    dr_matmul(pb, OFF[2] + (p * 2 + ph) * 256, 128,
                              A[2], (4 * jj + kiA) * WP + kjA, delta, 508,
                              start=(p == 0), stop=(p == 5),
                              final=(p == 5 and k % 2 == 1))
                if k % 2 == 1:
                    pe_marks.append(cnt['pe'])
            ev0 = pair_evac('act', prs[0], 2, A[3], 0, pe_marks[0], MS_A[3],
                            ('sh3', hcnt[3]) if hcnt[3] else None)
            ev1 = pair_evac('act', prs[1], 2, A[3], 8 * WP, pe_marks[1],
                            MS_A[3], None)
            st['l2_evac_h1'] = ev0
            st['l2_evac'] = ev1
            st['halo3'] = emit_halo(3, ev0, lambda: nc.sync.dma_start(
                A[3][0:112, 16 * WP: 18 * WP], A[3][16:128, 0:2 * WP]))

        def emit_l3(j, tau):
            st = S[j]
            prs = [take_pair(), take_pair()]
            wait('pe', 'sgp', MS_A[3])
            wait('pe', st['l2_evac_h1'][0], st['l2_evac_h1'][1])
            pe_marks = []
            for jj in range(4):
                if jj == 1:
                    wait('pe', st['l2_evac'][0], st['l2_evac'][1])
                if jj == 3:
                    wait('pe', st['halo3'][0], st['halo3'][1])
                pb = prs[jj // 2] * 1024 + (jj % 2) * 512
                for p, (tA, tB) in enumerate(PAIRS):
                    kiA, kjA = tA
                    delta = 128 if tB else 16
                    dr_matmul(pb, OFF[3] + (p * 2 + tau) * 256, 128,
                              A[3], (4 * jj + kiA) * WP + kjA, delta, 508,
                              start=(p == 0), stop=(p == 5),
                              final=(p == 5 and jj % 2 == 1))
                if jj % 2 == 1:
                    pe_marks.append(cnt['pe'])
            evs = []
            for pi in range(2):
                evs.append(pair_evac(
                    'dve', prs[pi], 2, A[4], tau * 16 * WP + pi * 1016,
                    pe_marks[pi], MS_A[4],
                    ('sh4', hcnt[4]) if (tau == 0 and pi == 0 and hcnt[4]) else None))
            st.setdefault('l3_evacs', {})[tau] = evs
            if tau == 0:
                st['halo4'] = emit_halo(4, evs[0], lambda: nc.sync.dma_start(
                    A[4][0:96, 32 * WP: 34 * WP], A[4][32:128, 0:2 * WP]))

        def emit_l4(j, h, tau):
            st = S[j]
            prs = [take_pair(), take_pair()]
            wait('pe', 'sgp', MS_A[4])
            for ev in st['l3_evacs'][tau]:
                wait('pe', ev[0], ev[1])
            pe_marks = []
            for jj in range(4):
                if jj == 3:
                    if tau == 0:
                        for ev in st['l3_evacs'][1]:
                            wait('pe', ev[0], ev[1])
                    else:
                        wait('pe', st['halo4'][0], st['halo4'][1])
                pb = prs[jj // 2] * 1024 + (jj % 2) * 512
                for p, (tA, tB) in enumerate(PAIRS):
                    kiA, kjA = tA
                    delta = 128 if tB else 16
                    dr_matmul(pb, OFF[4] + (p * 2 + h) * 256, 128,
                              A[4], (16 * tau + 4 * jj + kiA) * WP + kjA,
                              delta, 508,
                              start=(p == 0), stop=(p == 5),
                              final=(p == 5 and jj % 2 == 1))
                if jj % 2 == 1:
                    pe_marks.append(cnt['pe'])
            evs = []
            for pi in range(2):
                evs.append(pair_evac(
                    'act', prs[pi], 2, A[5],
                    h * SL5 + tau * 16 * WP + pi * 1016, pe_marks[pi], MS_A[5],
                    ('sh5', hcnt[5]) if (h == 0 and tau == 0 and pi == 0
                                         and hcnt[5]) else None))
            st.setdefault('l4_evacs', {})[(h, tau)] = evs
            if h == 1 and tau == 0:
                def mkh5():
                    sv = A[5][32:128, 0:SL5 + 2 * WP]
                    sv = dataclasses.replace(sv, ap=[sv.ap[0], [SL5, 2], [1, 2 * WP]])
                    dv = A[5][0:96, 32 * WP: SL5 + 34 * WP]
                    dv = dataclasses.replace(dv, ap=[dv.ap[0], [SL5, 2], [1, 2 * WP]])
                    return nc.sync.dma_start(dv, sv)
                wait('sp', st['l4_evacs'][(0, 0)][0][0],
                     st['l4_evacs'][(0, 0)][0][1])
                st['halo5'] = emit_halo(5, evs[0], mkh5)

        def emit_l5(j, tau):
            st = S[j]
            prs = [take_pair(), take_pair()]
            wait('pe', 'sgp', MS_A[5])
            if tau == 0:
                for ev in st['l4_evacs'][(1, 0)]:
                    wait('pe', ev[0], ev[1])
            for ev in st['l4_evacs'][(0, tau)] + st['l4_evacs'][(1, tau)]:
                wait('pe', ev[0], ev[1])
            pe_marks = []
            for jj in range(4):
                if jj == 3:
                    if tau == 0:
                        for ev in st['l4_evacs'][(1, 1)]:
                            wait('pe', ev[0], ev[1])
                    else:
                        wait('pe', st['halo5'][0], st['halo5'][1])
                pb = prs[jj // 2] * 1024 + (jj % 2) * 512
                for tap in range(9):
                    ki, kj = tap // 3, tap % 3
                    dr_matmul(pb, OFF[5] + tap * 256, 128,
                              A[5], (16 * tau + 4 * jj + ki) * WP + kj,
                              SL5, 508,
                              start=(tap == 0), stop=(tap == 8),
                              final=(tap == 8 and jj % 2 == 1))
                if jj % 2 == 1:
                    pe_marks.append(cnt['pe'])
            evs = []
            for pi in range(2):
                evs.append(pair_evac(
                    'dve', prs[pi], 2, A[6], tau * 16 * WP + pi * 1016,
                    pe_marks[pi], MS_A[6],
                    ('sh6', hcnt[6]) if (tau == 0 and pi == 0 and hcnt[6]) else None))
            st.setdefault('l5_evacs', {})[tau] = evs
            if tau == 0:
                st['halo6'] = emit_halo(6, evs[0], lambda: nc.sync.dma_start(
                    A[6][0:96, 32 * WP: 34 * WP], A[6][32:128, 0:2 * WP]))

        def emit_l6(j, tau):
            st = S[j]
            prs = [take_pair(), take_pair()]
            wait('pe', 'sgp', MS_A[6])
            for ev in st['l5_evacs'][tau]:
                wait('pe', ev[0], ev[1])
            pe_marks = []
            for jj in range(4):
                if jj == 3:
                    if tau == 0:
                        for ev in st['l5_evacs'][1]:
                            wait('pe', ev[0], ev[1])
                    else:
                        wait('pe', st['halo6'][0], st['halo6'][1])
                pb = prs[jj // 2] * 1024 + (jj % 2) * 512
                for p, (tA, tB) in enumerate(PAIRS):
                    kiA, kjA = tA
                    delta = 128 if tB else 16
                    dr_matmul(pb, OFF[6] + p * 32, 8,
                              A[6], (16 * tau + 4 * jj + kiA) * WP + kjA,
                              delta, 508,
                              start=(p == 0), stop=(p == 5),
                              final=(p == 5 and jj % 2 == 1))
                if jj % 2 == 1:
                    pe_marks.append(cnt['pe'])
            # evac per pair -> OUTB (fp8), then out-DMAs for this tau
            for pi in range(2):
                wait('act', 'spe', pe_marks[pi])
                if tau == 0 and pi == 0 and out_cnt[0]:
                    wait('act', 'sout', out_cnt[0])
                def mk6(pi=pi, tau=tau, prs=prs):
                    pb = prs[pi] * 1024
                    sv = PS[0:8, pb: pb + 512 + 508]
                    sv = dataclasses.replace(
                        sv, ap=[sv.ap[0], [512, 2], [127, 4], [1, 115]])
                    dv = OUTB[0:8, tau * 16 * 115 + pi * 8 * 115:
                              tau * 16 * 115 + pi * 8 * 115 + 8 * 115]
                    return nc.scalar.activation(dv, sv, AF.Sign)
                i = emit('act', mk6)
                cnt['act'] += 1
                inc(i, 'sact', 1)
                pair_free[prs[pi]] = ('sact', cnt['act'])
            # out-DMAs for rows [16*tau, 16*tau+nr)
            wait('sp', 'sact', cnt['act'])
            for g in range(4):
                nr = 16 if (tau == 0 or g < 3) else 3
                for c in range(2):
                    def mko(j=j, g=g, c=c, nr=nr, tau=tau):
                        sv = OUTB[2 * g + c: 2 * g + c + 1,
                                  tau * 1840: tau * 1840 + nr * 115]
                        dv = y[j, 0:1]
                        dv = dataclasses.replace(
                            dv, offset=dv.offset + c * 13225
                            + (32 * g + 16 * tau) * 115,
                            ap=[[1, nr * 115]])
                        return nc.sync.dma_start(dv, sv)
                    i = emit('sp', mko)
                    out_cnt[0] += 16
                    inc(i, 'sout', 16)

        # ---------------- schedule ----------------
        emit_in_dma(0)
        emit_in_dma(1)
        for j in range(NIMG):
            S[j]['l0_pe'] = S[j].get('l0_pe', [None] * 4)
        emit_l0_img0()
        for j in range(NIMG):
            if j + 2 < NIMG:
                emit_in_dma(j + 2)
            nxt = j + 1 if j + 1 < NIMG else None
            if nxt is not None:
                S[nxt]['l0_pe'] = [None] * 4
            emit_l1(j)
            if nxt is not None:
                emit_l0_chunk_steady(nxt, 0)
            emit_l2(j)
            if nxt is not None:
                emit_l0_chunk_steady(nxt, 1)
            emit_l3(j, 0)
            if nxt is not None:
                emit_l0_chunk_steady(nxt, 2)
            emit_l3(j, 1)
            emit_l4(j, 0, 0)
            emit_l4(j, 0, 1)
            if nxt is not None:
                emit_l0_chunk_steady(nxt, 3)
            emit_l4(j, 1, 0)
            emit_l4(j, 1, 1)
            emit_l5(j, 0)
            emit_l5(j, 1)
            emit_l6(j, 0)
            emit_l6(j, 1)
        return cnt

    with nc.Block() as block:
        @block.tensor
        def _(E):
            walk(E, 'pe')

        @block.scalar
        def _(E):
            walk(E, 'act')

        @block.vector
        def _(E):
            walk(E, 'dve')

        @block.gpsimd
        def _(E):
            walk(E, 'gp')

        @block.sync
        def _(E):
            walk(E, 'sp')

    for cm in reversed(ctxs):
        cm.__exit__(None, None, None)
    return nc


def round11(x):
    """Round fp32 array to 11-bit significand (RNE on low 13 mantissa bits)."""
    b = x.view(np.uint32).copy()
    low = b & np.uint32(0x1FFF)
    base = b & ~np.uint32(0x1FFF)
    rnd = (low > 0x1000) | ((low == 0x1000) & ((b >> 13) & 1).astype(bool))
    base = base + (rnd.astype(np.uint32) << 13)
    return base.view(np.float32)


def split_input(inp):
    """(N,3,256,256) fp32 -> (N,2,3,258,256): two 11-bit fp32r terms,
    rows zero-padded to 258 so band 31's halo loads in the main DMA."""
    t1 = round11(inp)
    t2 = round11((inp - t1).astype(np.float32))
    xs = np.zeros((inp.shape[0], 2, 3, 258, 256), np.float32)
    xs[:, 0, :, :256] = t1
    xs[:, 1, :, :256] = t2
    return xs


def pack_weights(ws):
    """ws: 7 raw arrays (cout, cin, 3, 3) -> (w0f fp32, wf8 fp8)."""
    import ml_dtypes
    sws = [np.sign(w).astype(np.float32) for w in ws]
    # L0: 32 bands x 3cin on 96 partitions -> 128 out (32 bands x 4)
    w0f = np.zeros((96, 9 * 128), np.float32)
    for tap in range(9):
        ki, kj = tap // 3, tap % 3
        blk = sws[0][:, :, ki, kj].T  # (cin, cout)
        for s in range(32):
            w0f[s * 3:s * 3 + 3, tap * 128 + s * 4: tap * 128 + s * 4 + 4] = blk
    wf8 = np.zeros((128, WF8_COLS), np.float32)
    # t2 layers: 1,2,3 (phases), 4 (cout halves), 6 (plain)
    for l, nph in ((1, 2), (2, 2), (3, 2)):
        cin, cout = CH[l]
        gin = G[l]
        M = 128
        for p, (tA, tB) in enumerate(PAIRS):
            for ph in range(nph):
                col = OFF[l] + (p * 2 + ph) * 256
                for i, tap in enumerate((tA, tB)):
                    if tap is None:
                        continue
                    ki, kj = tap
                    blk = sws[l][:, :, ki, kj].T  # (cin, cout)
                    for gp_ in range(gin // 2):
                        g = 2 * gp_ + ph
                        wf8[g * cin:(g + 1) * cin,
                            col + i * M + gp_ * cout: col + i * M + (gp_ + 1) * cout] = blk
    # L4: cout halves
    cin, cout = CH[4]
    for p, (tA, tB) in enumerate(PAIRS):
        for h in range(2):
            col = OFF[4] + (p * 2 + h) * 256
            for i, tap in enumerate((tA, tB)):
                if tap is None:
                    continue
                ki, kj = tap
                blk = sws[4][32 * h:32 * h + 32, :, ki, kj].T  # (32cin, 32cout)
                for g in range(4):
                    wf8[g * 32:(g + 1) * 32,
                        col + i * 128 + g * 32: col + i * 128 + (g + 1) * 32] = blk
    # L5: t1 k-split (slab i = channels 32i..32i+32)
    for tap in range(9):
        ki, kj = tap // 3, tap % 3
        col = OFF[5] + tap * 256
        for i in range(2):
            blk = sws[5][:, 32 * i:32 * i + 32, ki, kj].T  # (32cin-half, 32cout)
            for g in range(4):
                wf8[g * 32:(g + 1) * 32,
                    col + i * 128 + g * 32: col + i * 128 + (g + 1) * 32] = blk
    # L6: M=8 (ktile step padded to 16)
    for p, (tA, tB) in enumerate(PAIRS):
        col = OFF[6] + p * 32
        for i, tap in enumerate((tA, tB)):
            if tap is None:
                continue
            ki, kj = tap
            blk = sws[6][:, :, ki, kj].T  # (32, 2)
            for g in range(4):
                wf8[g * 32:(g + 1) * 32,
                    col + i * 16 + g * 2: col + i * 16 + (g + 1) * 2] = blk
    return w0f, wf8.astype(ml_dtypes.float8_e4m3fn)


LAST_RESULTS = None


def kernel(**inputs):
    global LAST_RESULTS
    from concourse.bass_utils import run_bass_kernel_spmd
    inp = np.asarray(inputs['inputs'], np.float32)
    ws = [np.asarray(inputs[f'w{i}']) for i in range(7)]
    w0f, wf8 = pack_weights(ws)
    nc = build_program()
    import ml_dtypes
    z8 = np.zeros((128, 9024), ml_dtypes.float8_e4m3fn)
    in_maps = []
    for c in range(8):
        xs = split_input(np.ascontiguousarray(inp[c * 8:(c + 1) * 8]))
        in_maps.append({'x': np.ascontiguousarray(xs),
                        'w0f': w0f, 'wf8': wf8, 'z8': z8})
    res = run_bass_kernel_spmd(nc, in_maps, core_ids=list(range(8)),
                               tmpdir=os.environ.get('KERNEL_TRACE_DIR') or None)
    LAST_RESULTS = res
    out = np.concatenate([np.asarray(res.results[c]['y'], np.float32)
                          for c in range(8)], axis=0)
    return out
